# revision 5
# baseline (speedup 1.0000x reference)
"""Bahdanau attention kernel for Trainium2, 8-core data-parallel. v3.

Shapes (hardcoded): features [256,225,1280] f32, hidden [256,256] f32,
W1 [1280,256], b1 [256], W2 [256,256], b2 [256], V [256,1], bV [1].
Output: context [256,1280] f32.

Sharding: batch dim split across 8 cores (32 per core); parameters
replicated; no collectives.

Per-core pipeline (batch pairs, software-pipelined):
  - SWDGE cast-DMA loads features natural [L,D] as bf16, full-partition
    DMAs only (SWDGE 16-increment rule); last batch's chunk B loads
    shifted (rows l=97..224).
  - feature transposes run as fp32 transposes of bf16 PAIRS (2 PE
    cyc/row but half the columns; f32r would truncate the packed bits).
  - step-1 matmul reads featX with a stride-4B bf16 parity AP; W1 is
    DMA'd pre-permuted to the matching (slab, parity) row order.
  - scoreT[u,l] = tanh(W1.T @ featT + bias) on ScalarE with per-batch
    bias = proj_hT[:,b] + b1 + b2.
  - logits are batched per 2 pairs: a [128,2] V-stationary with V in
    column q2 accumulates pair q2's logits into row q2 of one PSUM
    bank; softmax (exp/recip/scale) then runs on [2,450] rows, and the
    four attn transposes move 2 columns each instead of 1.
  - context: groups of 4 batches in one PSUM bank at partitions
    0/32/64/96 via tile_position col-tiling (phase A then B), drained
    by one wide ScalarE copy. softmax+ctx for group g issue right
    after head(2g+2) so the PE never waits on ACT.
"""

import numpy as np

import concourse.bass as bass
import concourse.bacc as bacc
import concourse.tile as tile
import concourse.mybir as mybir
from concourse import masks
from concourse.bass_utils import run_bass_kernel_spmd

B, L, D, H, U = 256, 225, 1280, 256, 256
NCORES = 8
BS = B // NCORES          # 32 batch items per core
L0, L1 = 128, L - 128     # 128 + 97
NSLAB = D // 256          # 5 slabs of 256 d-values (128 fp32 pairs)
F32 = mybir.dt.float32
F32R = mybir.dt.float32r
BF16 = mybir.dt.bfloat16
AF = mybir.ActivationFunctionType


def build_kernel():
    nc = bacc.Bacc("TRN2", target_bir_lowering=False, debug=False, num_devices=NCORES)

    # features/W1 are pre-cast to bf16 on the host inside kernel() --
    # the kernel computes in bf16 anyway, so this halves the HBM read
    # without changing numerics.
    feat = nc.dram_tensor("features", [BS, L, D], BF16, kind="ExternalInput").ap()
    hid = nc.dram_tensor("hidden", [BS, H], F32, kind="ExternalInput").ap()
    w1 = nc.dram_tensor("W1", [D, U], BF16, kind="ExternalInput").ap()
    b1 = nc.dram_tensor("b1", [U], F32, kind="ExternalInput").ap()
    w2 = nc.dram_tensor("W2", [H, U], F32, kind="ExternalInput").ap()
    b2 = nc.dram_tensor("b2", [U], F32, kind="ExternalInput").ap()
    v = nc.dram_tensor("V", [U, 1], F32, kind="ExternalInput").ap()
    nc.dram_tensor("bV", [1], F32, kind="ExternalInput")  # softmax-invariant
    ctx_out = nc.dram_tensor("context", [BS, D], F32, kind="ExternalOutput").ap()

    with tile.TileContext(nc) as tc:
        body(tc, feat, hid, w1, b1, w2, b2, v, ctx_out)
    nc.compile()
    return nc


def body(tc, feat, hid, w1, b1, w2, b2, v, ctx_out):
    nc = tc.nc
    from contextlib import ExitStack

    with ExitStack() as ctx:
        const = ctx.enter_context(tc.tile_pool(name="const", bufs=1))
        fnat_pool = ctx.enter_context(tc.tile_pool(name="fnat", bufs=9))
        featX_pool = ctx.enter_context(tc.tile_pool(name="featX", bufs=3))
        score_pool = ctx.enter_context(tc.tile_pool(name="score", bufs=3))
        small = ctx.enter_context(tc.tile_pool(name="small", bufs=2))
        att_pool = ctx.enter_context(tc.tile_pool(name="att", bufs=2))
        outst_pool = ctx.enter_context(tc.tile_pool(name="outst", bufs=2))
        # PSUM bank budget (8): trp*3 + scp0 + scp1 + lgp*2(shared w/ atp)
        # + cxp*1 = 8
        pp = ctx.enter_context(tc.tile_pool(name="pp", bufs=1, space="PSUM"))

        NPAIR = BS // 2
        st = {}
        feat_flat = feat.rearrange("b l d -> (b l) d")

        # ---- identity first: the first transposes wait on it, and the
        # gpsimd queue behind it fills with DMA descriptor generation ----
        ident = const.tile([128, 128], F32)
        masks.make_identity(nc, ident[:, :])
        ident_b4 = const.tile([4, 4], BF16)  # bf16 identity for attn transposes
        nc.vector.tensor_copy(ident_b4[:, :], ident[0:4, 0:4])

        def loads(pi):
            # every DMA covers 128 partitions (see module docstring)
            fnats = []
            for half in range(2):
                b = 2 * pi + half
                fnat2 = fnat_pool.tile([128, 2, D], BF16, tag=f"fnat{half}",
                                       name=f"fnat_{pi}_{half}")
                if b < BS - 1:
                    # one DMA for both L-chunks: row p, chunk t reads
                    # feat[b, t*128+p, :] (t=1, p>=97 overreads into the
                    # next batch's rows -- harmless garbage, in bounds)
                    nc.gpsimd.dma_start(
                        fnat2[:, :, :],
                        feat_flat[b * L:b * L + 256, :].rearrange(
                            "(t p) d -> p t d", p=128))
                else:
                    # last batch: chunk B shifted, row p = l 97+p
                    nc.gpsimd.dma_start(fnat2[:, 0, :], feat[b, 0:128, :])
                    nc.gpsimd.dma_start(fnat2[:, 1, :],
                                        feat_flat[BS * L - 128:BS * L, :])
                fnats.append((fnat2[:, 0, :], fnat2[:, 1, :]))
            st[("fnats", pi)] = fnats

        def loads0():
            # pair 0 on the critical path: split fine so the first
            # transposes start as soon as the first slabs land
            fnats = []
            for half in range(2):
                b = half
                fnat2 = fnat_pool.tile([128, 2, D], BF16, tag=f"fnat{half}",
                                       name=f"fnat_0_{half}")
                if half == 0:
                    nc.gpsimd.dma_start(fnat2[:, 0, 0:512], feat[b, 0:128, 0:512])
                    nc.gpsimd.dma_start(fnat2[:, 0, 512:D], feat[b, 0:128, 512:D])
                    nc.gpsimd.dma_start(fnat2[:, 1, :],
                                        feat_flat[b * L + 128:b * L + 256, :])
                else:
                    nc.gpsimd.dma_start(
                        fnat2[:, :, :],
                        feat_flat[b * L:b * L + 256, :].rearrange(
                            "(t p) d -> p t d", p=128))
                fnats.append((fnat2[:, 0, :], fnat2[:, 1, :]))
            st[("fnats", 0)] = fnats

        loads0()

        ones32 = const.tile([1, 32], F32)
        nc.vector.memset(ones32[:, :], 1.0)
        ones32_r = const.tile([1, 32], BF16)
        nc.vector.tensor_copy(ones32_r[:, :], ones32[:, :])

        # ---- W1, pre-permuted to (slab, parity) rows matching featX:
        # partition p, (slab k, parity s) holds W1[256k + 2p + s, :] ----
        w1_sb = const.tile([128, NSLAB, 2, U], BF16)
        nc.gpsimd.dma_start(
            w1_sb[:, :, :, :],
            w1.rearrange("(k p s) u -> p k s u", p=128, s=2))

        loads(1)

        w2_sb = const.tile([128, 2, U], BF16)   # [h_in_tile, h_tile, u]
        nc.gpsimd.dma_start(w2_sb[:, :, :], w2.rearrange("(k p) u -> p k u", p=128))

        loads(2)

        # ---- V natural [1, 256]: one contiguous descriptor ----
        v_nat = const.tile([1, U], F32)
        nc.scalar.dma_start(v_nat[:, :], v.rearrange("u o -> o u"))

        bsum = const.tile([1, U], BF16)         # b1 + b2 (both added pre-tanh)
        b1_sb = const.tile([1, U], F32)
        b2_sb = const.tile([1, U], F32)
        nc.scalar.dma_start(b1_sb[:, :], b1[None, :])
        nc.scalar.dma_start(b2_sb[:, :], b2[None, :])
        nc.vector.tensor_add(bsum[:, :], b1_sb[:, :], b2_sb[:, :])

        hid_nat = const.tile([32, H], F32)
        nc.scalar.dma_start(hid_nat[:, :], hid[:, :])

        loads(3)
        loads(4)

        hidT = const.tile([128, 2, BS], BF16)   # [h_in_tile, h_tile, b]
        projhT = const.tile([128, 2 * BS], F32)  # [u_in_tile, ut*32+b]
        v_sb = const.tile([128, 2], BF16)       # [u_in_tile, u_tile]
        vq = const.tile([128, 2, 2, 2], BF16)   # [u_in, ut, q2, col]

        def prolog_projh():
            # proj_hT [u, b] = W2.T @ hiddenT + (b1+b2); emitted after
            # head(0) so the PE works on feature transposes while the
            # weight DMAs land. Also builds v_sb / vq from v_nat on-chip.
            vp = pp.tile([128, 512], F32, tag="trp", bufs=3, name="vp")
            for ut in range(2):
                nc.tensor.transpose(
                    vp[:, ut:ut + 1], v_nat[0:1, ut * 128:(ut + 1) * 128],
                    ident[0:1, 0:1])
            nc.vector.tensor_copy(v_sb[:, :], vp[:, 0:2])
            nc.vector.memset(vq[:, :, :, :], 0.0)
            for ut in range(2):
                for q2 in range(2):
                    nc.vector.tensor_copy(vq[:, ut, q2, q2:q2 + 1],
                                          v_sb[:, ut:ut + 1])
            for hk in range(2):
                hp = pp.tile([128, 512], F32, tag="trp", bufs=3)
                nc.tensor.transpose(
                    hp[:, 0:32], hid_nat[0:32, hk * 128:(hk + 1) * 128],
                    ident[0:32, 0:32])
                nc.vector.tensor_copy(hidT[:, hk, :], hp[:, 0:32])
            for ut in range(2):
                php = pp.tile([128, 512], F32, tag="trp", bufs=3)
                for hk in range(2):
                    nc.tensor.matmul(
                        php[:, 0:32],
                        lhsT=w2_sb[:, hk, ut * 128:(ut + 1) * 128],
                        rhs=hidT[:, hk, :],
                        start=(hk == 0), stop=False)
                nc.tensor.matmul(
                    php[:, 0:32],
                    lhsT=bsum[0:1, ut * 128:(ut + 1) * 128],
                    rhs=ones32_r[0:1, :].opt(),
                    start=False, stop=True)
                nc.vector.tensor_copy(projhT[:, ut * BS:(ut + 1) * BS], php[:, 0:32])

        # ---- main loop over batch pairs, software-pipelined ----

        def head(pi):
            fnats = st[("fnats", pi)]
            # packed featT: fp32 lane p of slab k = d-pair (256k+2p, +1);
            # transposes run in f32r mode (1.5 PE cyc/row vs fp32 2.0)
            featX = featX_pool.tile([128, NSLAB, 2 * L], F32, tag="featX",
                                    name=f"featX_{pi}")
            frs = [(fA.bitcast(F32), fB.bitcast(F32)) for fA, fB in fnats]
            last = (pi == NPAIR - 1)
            for k in range(NSLAB):
                trp = pp.tile([128, 452], F32, tag="trp", bufs=3,
                              name=f"trp_{pi}_{k}")
                for half in range(2):
                    fA32, fB32 = frs[half]
                    o = 226 * half
                    nc.tensor.transpose(
                        trp[:, o:o + 128],
                        fA32[:, k * 128:(k + 1) * 128],
                        ident[:, :])
                    if last and half == 1:
                        # shifted chunk B: row p = l 97+p; cols o+97..o+224
                        # overlap chunk A's l 97..127 with identical values
                        nc.tensor.transpose(
                            trp[:, o + 97:o + 225],
                            fB32[:, k * 128:(k + 1) * 128],
                            ident[:, :])
                    else:
                        nc.tensor.transpose(
                            trp[:, o + 128:o + 128 + L1 + 1],
                            fB32[0:L1 + 1, k * 128:(k + 1) * 128],
                            ident[0:L1 + 1, 0:L1 + 1])
                nc.vector.tensor_copy(
                    featX[:, k, :],
                    trp.rearrange("p (h x) -> p h x", h=2)[:, :, 0:L])

            # bf16 parity view: [p, slab, l, s]
            featXb = featX[:, :, :].bitcast(BF16).rearrange(
                "p k (l s) -> p k l s", s=2)
            scps = []
            for ut in range(2):
                scp = pp.tile([128, 512], F32, tag=f"scp{ut}", bufs=1,
                              name=f"scp_{pi}_{ut}")
                for k in range(NSLAB):
                    for s in range(2):
                        nc.tensor.matmul(
                            scp[:, 0:2 * L],
                            lhsT=w1_sb[:, k, s, ut * 128:(ut + 1) * 128],
                            rhs=featXb[:, k, :, s],
                            start=(k == 0 and s == 0),
                            stop=(k == NSLAB - 1 and s == 1))
                scps.append(scp)
            st[("scps", pi)] = scps

        def tanh_part(pi):
            scps = st.pop(("scps", pi))
            score_sb = score_pool.tile([128, 2, 2 * L], BF16, tag="score_sb",
                                       name=f"score_{pi}")
            for ut in range(2):
                for half in range(2):
                    b = 2 * pi + half
                    nc.scalar.activation(
                        score_sb[:, ut, half * L:(half + 1) * L],
                        scps[ut][:, half * L:(half + 1) * L],
                        AF.Tanh,
                        bias=projhT[:, ut * BS + b:ut * BS + b + 1])
            st[("score", pi)] = score_sb

        def logits(pi):
            # rows 0/1 of one psum bank per 2-pair group, accumulated
            q2 = pi % 2
            if q2 == 0:
                lgp = pp.tile([128, 512], F32, tag="lgp", bufs=2,
                              name=f"lgp_{pi // 2}")
                st[("lgp", pi // 2)] = lgp
            else:
                lgp = st[("lgp", pi // 2)]
            score_sb = st.pop(("score", pi))
            for ut in range(2):
                nc.tensor.matmul(
                    lgp[0:2, 0:2 * L],
                    lhsT=vq[:, ut, q2, :],
                    rhs=score_sb[:, ut, :],
                    start=(q2 == 0 and ut == 0), stop=(q2 == 1 and ut == 1),
                    skip_group_check=True)

        def softmax_act(g):
            # exp/recip/scale for batches 4g..4g+3 on ACT+DVE ([2,450] rows)
            lgp = st.pop(("lgp", g))
            expl = small.tile([2, 2 * L], BF16, tag="expl", name=f"expl_{g}")
            esum = small.tile([2, 2], F32, tag="esum", name=f"esum_{g}")
            for half in range(2):
                nc.scalar.activation(
                    expl[0:2, half * L:(half + 1) * L],
                    lgp[0:2, half * L:(half + 1) * L], AF.Exp,
                    accum_out=esum[0:2, half:half + 1])
            rsum = small.tile([2, 2], F32, tag="rsum", name=f"rsum_{g}")
            nc.vector.reciprocal(rsum[:, :], esum[:, :])
            attn = small.tile([2, 2 * L], BF16, tag="attn", name=f"attn_{g}")
            for half in range(2):
                nc.vector.tensor_scalar_mul(
                    attn[0:2, half * L:(half + 1) * L],
                    expl[0:2, half * L:(half + 1) * L],
                    rsum[0:2, half:half + 1])
            st[("attn", g)] = attn

        def transp_flush(g):
            # attn transposes (2 cols per transpose) + ctx flush for the
            # 4 batches of group g
            attn = st.pop(("attn", g))
            last = (g == NPAIR // 2 - 1)
            # atp shares the lgp psum ring: same row bytes (bf16, 2x cols)
            atp = pp.tile([128, 1024], BF16, tag="lgp", bufs=2,
                          name=f"atp_{g}")
            nc.tensor.transpose(atp[0:128, 0:2], attn[0:2, 0:128],
                                ident_b4[0:2, 0:2])
            nc.tensor.transpose(atp[0:128, 2:4], attn[0:2, L:L + 128],
                                ident_b4[0:2, 0:2])
            nc.tensor.transpose(atp[0:97, 4:6], attn[0:2, 128:L],
                                ident_b4[0:2, 0:2])
            if last:
                # b31 (odd row 1) has shifted chunk B: stage a [2,128] row
                # pair -- row 0 (b29) at cols 0:97, row 1 (b31) shifted to
                # cols 31:128 -- then one 2-col transpose covers both.
                a31 = small.tile([2, 128], BF16, tag="a31", name="a31")
                nc.vector.memset(a31[:, :], 0.0)
                nc.vector.tensor_copy(a31[0:2, 31:128], attn[0:2, L + 128:2 * L])
                nc.vector.memset(a31[0:1, 97:128], 0.0)
                nc.vector.tensor_copy(a31[0:1, 0:97], attn[0:1, L + 128:2 * L])
                nc.tensor.transpose(atp[0:128, 6:8], a31[0:2, 0:128],
                                    ident_b4[0:2, 0:2])
            else:
                nc.tensor.transpose(atp[0:97, 6:8], attn[0:2, L + 128:2 * L],
                                    ident_b4[0:2, 0:2])

            attnT = att_pool.tile([128, 4, 64], BF16, tag="attnT",
                                  name=f"attnT_{g}")
            nc.vector.memset(attnT[:, :, :], 0.0)
            av = attnT.rearrange("p (j e) c -> p j e c", e=2)
            nc.vector.tensor_copy(av[:, :, 0, 0], atp[0:128, 0:2])
            nc.vector.tensor_copy(av[:, :, 1, 0], atp[0:128, 2:4])
            nc.vector.tensor_copy(av[0:97, :, 0, 32], atp[0:97, 4:6])
            if last:
                nc.vector.tensor_copy(av[:, :, 1, 32], atp[0:128, 6:8])
            else:
                nc.vector.tensor_copy(av[0:97, :, 1, 32], atp[0:97, 6:8])

            fn0 = st.pop(("fnats", 2 * g))
            fn1 = st.pop(("fnats", 2 * g + 1))
            group = []
            for j, (fnA, fnB) in enumerate(fn0 + fn1):
                bK = 128 if (last and j == 3) else L1
                group.append((4 * g + j, attnT, j, fnA, fnB, bK))

            # 4 batches -> one PSUM bank at partitions 0/32/64/96.
            out4 = outst_pool.tile([128, D], F32, tag="out_stage",
                                   name=f"outst_{group[0][0]}")
            b0 = group[0][0]
            for doff, dw in ((0, 512), (512, 512), (1024, 256)):
                cxp = pp.tile([128, 512], F32, tag="cxp", bufs=1,
                              name=f"cxp_{group[0][0]}_{doff}")
                for q, (b, attnT_, j, fnatA, fnatB, bK) in enumerate(group):
                    nc.tensor.matmul(
                        cxp[32 * q:32 * q + 32, 0:dw],
                        lhsT=attnT_[0:128, j, 0:32],
                        rhs=fnatA[:, doff:doff + dw],
                        start=True, stop=False,
                        skip_group_check=True,
                        tile_position=(0, 32 * q))
                for q, (b, attnT_, j, fnatA, fnatB, bK) in enumerate(group):
                    nc.tensor.matmul(
                        cxp[32 * q:32 * q + 32, 0:dw],
                        lhsT=attnT_[0:bK, j, 32:64],
                        rhs=fnatB[0:bK, doff:doff + dw],
                        start=False, stop=True,
                        skip_group_check=True,
                        tile_position=(0, 32 * q))
                nc.scalar.copy(out4[0:97, doff:doff + dw],
                               cxp[0:97, 0:dw])
                if last:
                    # final group: store per chunk so the kernel-ending DMA
                    # only covers the last 256 columns (sync queue is idle)
                    nc.sync.dma_start(
                        ctx_out[b0:b0 + 4, doff:doff + dw],
                        out4.rearrange("(q r) d -> q r d", r=32)[:, 0,
                                                                 doff:doff + dw])
            if not last:
                nc.sync.dma_start(
                    ctx_out[b0:b0 + 4, :],
                    out4.rearrange("(q r) d -> q r d", r=32)[:, 0, :])

        for pi in range(NPAIR):
            if pi + 5 < NPAIR:
                loads(pi + 5)
            head(pi)
            if pi == 0:
                prolog_projh()
            if pi % 2 == 0 and pi >= 2 and pi < NPAIR - 2:
                # softmax + ctx for the previous group: ACT ops issue
                # before tanh(pi) so the PE transposes/ctx never stall
                softmax_act(pi // 2 - 1)
                transp_flush(pi // 2 - 1)
            if pi == NPAIR - 1:
                # second-to-last group deferred to here: its ctx matmuls
                # keep the PE busy through tanh(15)'s ACT latency
                softmax_act(NPAIR // 2 - 2)
                transp_flush(NPAIR // 2 - 2)
            tanh_part(pi)
            logits(pi)
        softmax_act(NPAIR // 2 - 1)
        transp_flush(NPAIR // 2 - 1)


def _enable_jax_cache():
    try:
        import jax
        jax.config.update("jax_compilation_cache_dir", "/tmp/jax_neff_cache")
        jax.config.update("jax_persistent_cache_min_entry_size_bytes", 0)
        jax.config.update("jax_persistent_cache_min_compile_time_secs", 0)
    except Exception:
        pass


_enable_jax_cache()

_CACHE = {}


def _get_nc():
    if "nc" not in _CACHE:
        _CACHE["nc"] = build_kernel()
    return _CACHE["nc"]


def _run(inputs, trace=False):
    import ml_dtypes
    nc = _get_nc()
    feat_bf = np.ascontiguousarray(
        inputs["features"].astype(ml_dtypes.bfloat16))
    w1_bf = np.ascontiguousarray(inputs["W1"].astype(ml_dtypes.bfloat16))
    in_maps = []
    for c in range(NCORES):
        sl = slice(c * BS, (c + 1) * BS)
        in_maps.append({
            "features": feat_bf[sl],
            "hidden": np.ascontiguousarray(inputs["hidden"][sl]),
            "W1": w1_bf,
            "b1": np.ascontiguousarray(inputs["b1"]),
            "W2": np.ascontiguousarray(inputs["W2"]),
            "b2": np.ascontiguousarray(inputs["b2"]),
            "V": np.ascontiguousarray(inputs["V"]),
            "bV": np.ascontiguousarray(inputs["bV"]),
        })
    res = run_bass_kernel_spmd(nc, in_maps, core_ids=list(range(NCORES)),
                               trace=trace)
    out = np.concatenate([rr["context"] for rr in res.results], axis=0)
    return out, res


def kernel(**inputs):
    out, _ = _run(inputs, trace=False)
    return out


# revision 6
# speedup vs baseline: 1.0069x; 1.0069x over previous
"""Bahdanau attention kernel for Trainium2, 8-core data-parallel. v3.

Shapes (hardcoded): features [256,225,1280] f32, hidden [256,256] f32,
W1 [1280,256], b1 [256], W2 [256,256], b2 [256], V [256,1], bV [1].
Output: context [256,1280] f32.

Sharding: batch dim split across 8 cores (32 per core); parameters
replicated; no collectives.

Per-core pipeline (batch pairs, software-pipelined):
  - SWDGE cast-DMA loads features natural [L,D] as bf16, full-partition
    DMAs only (SWDGE 16-increment rule); last batch's chunk B loads
    shifted (rows l=97..224).
  - feature transposes run as fp32 transposes of bf16 PAIRS (2 PE
    cyc/row but half the columns; f32r would truncate the packed bits).
  - step-1 matmul reads featX with a stride-4B bf16 parity AP; W1 is
    DMA'd pre-permuted to the matching (slab, parity) row order.
  - scoreT[u,l] = tanh(W1.T @ featT + bias) on ScalarE with per-batch
    bias = proj_hT[:,b] + b1 + b2.
  - logits are batched per 2 pairs: a [128,2] V-stationary with V in
    column q2 accumulates pair q2's logits into row q2 of one PSUM
    bank; softmax (exp/recip/scale) then runs on [2,450] rows, and the
    four attn transposes move 2 columns each instead of 1.
  - context: groups of 4 batches in one PSUM bank at partitions
    0/32/64/96 via tile_position col-tiling (phase A then B), drained
    by one wide ScalarE copy. softmax+ctx for group g issue right
    after head(2g+2) so the PE never waits on ACT.
"""

import numpy as np

import concourse.bass as bass
import concourse.bacc as bacc
import concourse.tile as tile
import concourse.mybir as mybir
from concourse import masks
from concourse.bass_utils import run_bass_kernel_spmd

B, L, D, H, U = 256, 225, 1280, 256, 256
NCORES = 8
BS = B // NCORES          # 32 batch items per core
L0, L1 = 128, L - 128     # 128 + 97
NSLAB = D // 256          # 5 slabs of 256 d-values (128 fp32 pairs)
F32 = mybir.dt.float32
F32R = mybir.dt.float32r
BF16 = mybir.dt.bfloat16
AF = mybir.ActivationFunctionType


def build_kernel():
    nc = bacc.Bacc("TRN2", target_bir_lowering=False, debug=False, num_devices=NCORES)

    # features/W1 are pre-cast to bf16 on the host inside kernel() --
    # the kernel computes in bf16 anyway, so this halves the HBM read
    # without changing numerics.
    feat = nc.dram_tensor("features", [BS, L, D], BF16, kind="ExternalInput").ap()
    hid = nc.dram_tensor("hidden", [BS, H], F32, kind="ExternalInput").ap()
    w1 = nc.dram_tensor("W1", [D, U], BF16, kind="ExternalInput").ap()
    b1 = nc.dram_tensor("b1", [U], F32, kind="ExternalInput").ap()
    w2 = nc.dram_tensor("W2", [H, U], F32, kind="ExternalInput").ap()
    b2 = nc.dram_tensor("b2", [U], F32, kind="ExternalInput").ap()
    v = nc.dram_tensor("V", [U, 1], F32, kind="ExternalInput").ap()
    nc.dram_tensor("bV", [1], F32, kind="ExternalInput")  # softmax-invariant
    ctx_out = nc.dram_tensor("context", [BS, D], F32, kind="ExternalOutput").ap()

    with tile.TileContext(nc) as tc:
        body(tc, feat, hid, w1, b1, w2, b2, v, ctx_out)
    nc.compile()
    return nc


def body(tc, feat, hid, w1, b1, w2, b2, v, ctx_out):
    nc = tc.nc
    from contextlib import ExitStack

    with ExitStack() as ctx:
        const = ctx.enter_context(tc.tile_pool(name="const", bufs=1))
        fnat_pool = ctx.enter_context(tc.tile_pool(name="fnat", bufs=9))
        featX_pool = ctx.enter_context(tc.tile_pool(name="featX", bufs=3))
        score_pool = ctx.enter_context(tc.tile_pool(name="score", bufs=3))
        small = ctx.enter_context(tc.tile_pool(name="small", bufs=2))
        att_pool = ctx.enter_context(tc.tile_pool(name="att", bufs=2))
        outst_pool = ctx.enter_context(tc.tile_pool(name="outst", bufs=2))
        # PSUM bank budget (8): trp*3 + scp0 + scp1 + lgp*2(shared w/ atp)
        # + cxp*1 = 8
        pp = ctx.enter_context(tc.tile_pool(name="pp", bufs=1, space="PSUM"))

        NPAIR = BS // 2
        st = {}
        feat_flat = feat.rearrange("b l d -> (b l) d")

        # ---- identity first: the first transposes wait on it, and the
        # gpsimd queue behind it fills with DMA descriptor generation ----
        ident = const.tile([128, 128], F32)
        masks.make_identity(nc, ident[:, :])
        ident_b4 = const.tile([4, 4], BF16)  # bf16 identity for attn transposes
        nc.vector.tensor_copy(ident_b4[:, :], ident[0:4, 0:4])

        def loads(pi):
            # every DMA covers 128 partitions (see module docstring)
            fnats = []
            for half in range(2):
                b = 2 * pi + half
                fnat2 = fnat_pool.tile([128, 2, D], BF16, tag=f"fnat{half}",
                                       name=f"fnat_{pi}_{half}")
                if b < BS - 1:
                    # one DMA for both L-chunks: row p, chunk t reads
                    # feat[b, t*128+p, :] (t=1, p>=97 overreads into the
                    # next batch's rows -- harmless garbage, in bounds)
                    nc.gpsimd.dma_start(
                        fnat2[:, :, :],
                        feat_flat[b * L:b * L + 256, :].rearrange(
                            "(t p) d -> p t d", p=128))
                else:
                    # last batch: chunk B shifted, row p = l 97+p
                    nc.gpsimd.dma_start(fnat2[:, 0, :], feat[b, 0:128, :])
                    nc.gpsimd.dma_start(fnat2[:, 1, :],
                                        feat_flat[BS * L - 128:BS * L, :])
                fnats.append((fnat2[:, 0, :], fnat2[:, 1, :]))
            st[("fnats", pi)] = fnats

        def loads0():
            # pair 0 on the critical path: split fine so the first
            # transposes start as soon as the first slabs land
            fnats = []
            for half in range(2):
                b = half
                fnat2 = fnat_pool.tile([128, 2, D], BF16, tag=f"fnat{half}",
                                       name=f"fnat_0_{half}")
                if half == 0:
                    nc.gpsimd.dma_start(fnat2[:, 0, 0:512], feat[b, 0:128, 0:512])
                    nc.gpsimd.dma_start(fnat2[:, 0, 512:D], feat[b, 0:128, 512:D])
                    nc.gpsimd.dma_start(fnat2[:, 1, :],
                                        feat_flat[b * L + 128:b * L + 256, :])
                else:
                    nc.gpsimd.dma_start(
                        fnat2[:, :, :],
                        feat_flat[b * L:b * L + 256, :].rearrange(
                            "(t p) d -> p t d", p=128))
                fnats.append((fnat2[:, 0, :], fnat2[:, 1, :]))
            st[("fnats", 0)] = fnats

        loads0()

        ones32 = const.tile([1, 32], F32)
        nc.vector.memset(ones32[:, :], 1.0)
        ones32_r = const.tile([1, 32], BF16)
        nc.vector.tensor_copy(ones32_r[:, :], ones32[:, :])

        # ---- W1, pre-permuted to (slab, parity) rows matching featX:
        # partition p, (slab k, parity s) holds W1[256k + 2p + s, :] ----
        w1_sb = const.tile([128, NSLAB, 2, U], BF16)
        nc.scalar.dma_start(
            w1_sb[:, :, :, :],
            w1.rearrange("(k p s) u -> p k s u", p=128, s=2))

        loads(1)

        w2_sb = const.tile([128, 2, U], BF16)   # [h_in_tile, h_tile, u]
        nc.gpsimd.dma_start(w2_sb[:, :, :], w2.rearrange("(k p) u -> p k u", p=128))

        loads(2)

        # ---- V natural [1, 256]: one contiguous descriptor ----
        v_nat = const.tile([1, U], F32)
        nc.scalar.dma_start(v_nat[:, :], v.rearrange("u o -> o u"))

        bsum = const.tile([1, U], BF16)         # b1 + b2 (both added pre-tanh)
        b1_sb = const.tile([1, U], F32)
        b2_sb = const.tile([1, U], F32)
        nc.scalar.dma_start(b1_sb[:, :], b1[None, :])
        nc.scalar.dma_start(b2_sb[:, :], b2[None, :])
        nc.vector.tensor_add(bsum[:, :], b1_sb[:, :], b2_sb[:, :])

        hid_nat = const.tile([32, H], F32)
        nc.scalar.dma_start(hid_nat[:, :], hid[:, :])

        loads(3)
        loads(4)

        hidT = const.tile([128, 2, BS], BF16)   # [h_in_tile, h_tile, b]
        projhT = const.tile([128, 2 * BS], F32)  # [u_in_tile, ut*32+b]
        v_sb = const.tile([128, 2], BF16)       # [u_in_tile, u_tile]
        vq = const.tile([128, 2, 2, 2], BF16)   # [u_in, ut, q2, col]

        def prolog_projh():
            # proj_hT [u, b] = W2.T @ hiddenT + (b1+b2); emitted after
            # head(0) so the PE works on feature transposes while the
            # weight DMAs land. Also builds v_sb / vq from v_nat on-chip.
            vp = pp.tile([128, 512], F32, tag="trp", bufs=3, name="vp")
            for ut in range(2):
                nc.tensor.transpose(
                    vp[:, ut:ut + 1], v_nat[0:1, ut * 128:(ut + 1) * 128],
                    ident[0:1, 0:1])
            nc.vector.tensor_copy(v_sb[:, :], vp[:, 0:2])
            nc.vector.memset(vq[:, :, :, :], 0.0)
            for ut in range(2):
                for q2 in range(2):
                    nc.vector.tensor_copy(vq[:, ut, q2, q2:q2 + 1],
                                          v_sb[:, ut:ut + 1])
            for hk in range(2):
                hp = pp.tile([128, 512], F32, tag="trp", bufs=3)
                nc.tensor.transpose(
                    hp[:, 0:32], hid_nat[0:32, hk * 128:(hk + 1) * 128],
                    ident[0:32, 0:32])
                nc.vector.tensor_copy(hidT[:, hk, :], hp[:, 0:32])
            for ut in range(2):
                php = pp.tile([128, 512], F32, tag="trp", bufs=3)
                for hk in range(2):
                    nc.tensor.matmul(
                        php[:, 0:32],
                        lhsT=w2_sb[:, hk, ut * 128:(ut + 1) * 128],
                        rhs=hidT[:, hk, :],
                        start=(hk == 0), stop=False)
                nc.tensor.matmul(
                    php[:, 0:32],
                    lhsT=bsum[0:1, ut * 128:(ut + 1) * 128],
                    rhs=ones32_r[0:1, :].opt(),
                    start=False, stop=True)
                nc.vector.tensor_copy(projhT[:, ut * BS:(ut + 1) * BS], php[:, 0:32])

        # ---- main loop over batch pairs, software-pipelined ----

        def head0_split():
            # pair 0: process per batch so step-1 on batch 0 starts after
            # only half the pair's bytes (+W1) have landed
            fnats = st[("fnats", 0)]
            featX = featX_pool.tile([128, NSLAB, 2 * L], F32, tag="featX",
                                    name="featX_0")
            frs = [(fA.bitcast(F32), fB.bitcast(F32)) for fA, fB in fnats]
            scps = [pp.tile([128, 512], F32, tag=f"scp{ut}", bufs=1,
                            name=f"scp_0_{ut}") for ut in range(2)]
            featXb = featX[:, :, :].bitcast(BF16).rearrange(
                "p k (l s) -> p k l s", s=2)
            for half in range(2):
                fA32, fB32 = frs[half]
                for k in range(NSLAB):
                    trp = pp.tile([128, 452], F32, tag="trp", bufs=3,
                                  name=f"trp_0_{half}_{k}")
                    nc.tensor.transpose(trp[:, 0:128],
                                        fA32[:, k * 128:(k + 1) * 128],
                                        ident[:, :])
                    nc.tensor.transpose(trp[:, 128:128 + L1 + 1],
                                        fB32[0:L1 + 1, k * 128:(k + 1) * 128],
                                        ident[0:L1 + 1, 0:L1 + 1])
                    nc.vector.tensor_copy(featX[:, k, half * L:(half + 1) * L],
                                          trp[:, 0:L])
                for ut in range(2):
                    for k in range(NSLAB):
                        for s in range(2):
                            nc.tensor.matmul(
                                scps[ut][:, half * L:(half + 1) * L],
                                lhsT=w1_sb[:, k, s, ut * 128:(ut + 1) * 128],
                                rhs=featXb[:, k, half * L:(half + 1) * L, s],
                                start=(k == 0 and s == 0),
                                stop=(k == NSLAB - 1 and s == 1),
                                skip_group_check=True)
            st[("scps", 0)] = scps

        def head(pi):
            fnats = st[("fnats", pi)]
            # packed featT: fp32 lane p of slab k = d-pair (256k+2p, +1);
            # transposes run in f32r mode (1.5 PE cyc/row vs fp32 2.0)
            featX = featX_pool.tile([128, NSLAB, 2 * L], F32, tag="featX",
                                    name=f"featX_{pi}")
            frs = [(fA.bitcast(F32), fB.bitcast(F32)) for fA, fB in fnats]
            last = (pi == NPAIR - 1)
            for k in range(NSLAB):
                trp = pp.tile([128, 452], F32, tag="trp", bufs=3,
                              name=f"trp_{pi}_{k}")
                for half in range(2):
                    fA32, fB32 = frs[half]
                    o = 226 * half
                    nc.tensor.transpose(
                        trp[:, o:o + 128],
                        fA32[:, k * 128:(k + 1) * 128],
                        ident[:, :])
                    if last and half == 1:
                        # shifted chunk B: row p = l 97+p; cols o+97..o+224
                        # overlap chunk A's l 97..127 with identical values
                        nc.tensor.transpose(
                            trp[:, o + 97:o + 225],
                            fB32[:, k * 128:(k + 1) * 128],
                            ident[:, :])
                    else:
                        nc.tensor.transpose(
                            trp[:, o + 128:o + 128 + L1 + 1],
                            fB32[0:L1 + 1, k * 128:(k + 1) * 128],
                            ident[0:L1 + 1, 0:L1 + 1])
                nc.vector.tensor_copy(
                    featX[:, k, :],
                    trp.rearrange("p (h x) -> p h x", h=2)[:, :, 0:L])

            # bf16 parity view: [p, slab, l, s]
            featXb = featX[:, :, :].bitcast(BF16).rearrange(
                "p k (l s) -> p k l s", s=2)
            scps = []
            for ut in range(2):
                scp = pp.tile([128, 512], F32, tag=f"scp{ut}", bufs=1,
                              name=f"scp_{pi}_{ut}")
                for k in range(NSLAB):
                    for s in range(2):
                        nc.tensor.matmul(
                            scp[:, 0:2 * L],
                            lhsT=w1_sb[:, k, s, ut * 128:(ut + 1) * 128],
                            rhs=featXb[:, k, :, s],
                            start=(k == 0 and s == 0),
                            stop=(k == NSLAB - 1 and s == 1))
                scps.append(scp)
            st[("scps", pi)] = scps

        def tanh_part(pi):
            scps = st.pop(("scps", pi))
            score_sb = score_pool.tile([128, 2, 2 * L], BF16, tag="score_sb",
                                       name=f"score_{pi}")
            for ut in range(2):
                for half in range(2):
                    b = 2 * pi + half
                    nc.scalar.activation(
                        score_sb[:, ut, half * L:(half + 1) * L],
                        scps[ut][:, half * L:(half + 1) * L],
                        AF.Tanh,
                        bias=projhT[:, ut * BS + b:ut * BS + b + 1])
            st[("score", pi)] = score_sb

        def logits(pi):
            # rows 0/1 of one psum bank per 2-pair group, accumulated
            q2 = pi % 2
            if q2 == 0:
                lgp = pp.tile([128, 512], F32, tag="lgp", bufs=2,
                              name=f"lgp_{pi // 2}")
                st[("lgp", pi // 2)] = lgp
            else:
                lgp = st[("lgp", pi // 2)]
            score_sb = st.pop(("score", pi))
            for ut in range(2):
                nc.tensor.matmul(
                    lgp[0:2, 0:2 * L],
                    lhsT=vq[:, ut, q2, :],
                    rhs=score_sb[:, ut, :],
                    start=(q2 == 0 and ut == 0), stop=(q2 == 1 and ut == 1),
                    skip_group_check=True)

        def softmax_act(g):
            # exp/recip/scale for batches 4g..4g+3 on ACT+DVE ([2,450] rows)
            lgp = st.pop(("lgp", g))
            expl = small.tile([2, 2 * L], BF16, tag="expl", name=f"expl_{g}")
            esum = small.tile([2, 2], F32, tag="esum", name=f"esum_{g}")
            for half in range(2):
                nc.scalar.activation(
                    expl[0:2, half * L:(half + 1) * L],
                    lgp[0:2, half * L:(half + 1) * L], AF.Exp,
                    accum_out=esum[0:2, half:half + 1])
            rsum = small.tile([2, 2], F32, tag="rsum", name=f"rsum_{g}")
            nc.vector.reciprocal(rsum[:, :], esum[:, :])
            attn = small.tile([2, 2 * L], BF16, tag="attn", name=f"attn_{g}")
            for half in range(2):
                nc.vector.tensor_scalar_mul(
                    attn[0:2, half * L:(half + 1) * L],
                    expl[0:2, half * L:(half + 1) * L],
                    rsum[0:2, half:half + 1])
            st[("attn", g)] = attn

        def transp_flush(g):
            # attn transposes (2 cols per transpose) + ctx flush for the
            # 4 batches of group g
            attn = st.pop(("attn", g))
            last = (g == NPAIR // 2 - 1)
            # atp shares the lgp psum ring: same row bytes (bf16, 2x cols)
            atp = pp.tile([128, 1024], BF16, tag="lgp", bufs=2,
                          name=f"atp_{g}")
            nc.tensor.transpose(atp[0:128, 0:2], attn[0:2, 0:128],
                                ident_b4[0:2, 0:2])
            nc.tensor.transpose(atp[0:128, 2:4], attn[0:2, L:L + 128],
                                ident_b4[0:2, 0:2])
            nc.tensor.transpose(atp[0:97, 4:6], attn[0:2, 128:L],
                                ident_b4[0:2, 0:2])
            if last:
                # b31 (odd row 1) has shifted chunk B: stage a [2,128] row
                # pair -- row 0 (b29) at cols 0:97, row 1 (b31) shifted to
                # cols 31:128 -- then one 2-col transpose covers both.
                a31 = small.tile([2, 128], BF16, tag="a31", name="a31")
                nc.vector.memset(a31[:, :], 0.0)
                nc.vector.tensor_copy(a31[0:2, 31:128], attn[0:2, L + 128:2 * L])
                nc.vector.memset(a31[0:1, 97:128], 0.0)
                nc.vector.tensor_copy(a31[0:1, 0:97], attn[0:1, L + 128:2 * L])
                nc.tensor.transpose(atp[0:128, 6:8], a31[0:2, 0:128],
                                    ident_b4[0:2, 0:2])
            else:
                nc.tensor.transpose(atp[0:97, 6:8], attn[0:2, L + 128:2 * L],
                                    ident_b4[0:2, 0:2])

            attnT = att_pool.tile([128, 4, 64], BF16, tag="attnT",
                                  name=f"attnT_{g}")
            nc.vector.memset(attnT[:, :, :], 0.0)
            av = attnT.rearrange("p (j e) c -> p j e c", e=2)
            nc.vector.tensor_copy(av[:, :, 0, 0], atp[0:128, 0:2])
            nc.vector.tensor_copy(av[:, :, 1, 0], atp[0:128, 2:4])
            nc.vector.tensor_copy(av[0:97, :, 0, 32], atp[0:97, 4:6])
            if last:
                nc.vector.tensor_copy(av[:, :, 1, 32], atp[0:128, 6:8])
            else:
                nc.vector.tensor_copy(av[0:97, :, 1, 32], atp[0:97, 6:8])

            fn0 = st.pop(("fnats", 2 * g))
            fn1 = st.pop(("fnats", 2 * g + 1))
            group = []
            for j, (fnA, fnB) in enumerate(fn0 + fn1):
                bK = 128 if (last and j == 3) else L1
                group.append((4 * g + j, attnT, j, fnA, fnB, bK))

            # 4 batches -> one PSUM bank at partitions 0/32/64/96.
            out4 = outst_pool.tile([128, D], F32, tag="out_stage",
                                   name=f"outst_{group[0][0]}")
            b0 = group[0][0]
            for doff, dw in ((0, 512), (512, 512), (1024, 256)):
                cxp = pp.tile([128, 512], F32, tag="cxp", bufs=1,
                              name=f"cxp_{group[0][0]}_{doff}")
                for q, (b, attnT_, j, fnatA, fnatB, bK) in enumerate(group):
                    nc.tensor.matmul(
                        cxp[32 * q:32 * q + 32, 0:dw],
                        lhsT=attnT_[0:128, j, 0:32],
                        rhs=fnatA[:, doff:doff + dw],
                        start=True, stop=False,
                        skip_group_check=True,
                        tile_position=(0, 32 * q))
                for q, (b, attnT_, j, fnatA, fnatB, bK) in enumerate(group):
                    nc.tensor.matmul(
                        cxp[32 * q:32 * q + 32, 0:dw],
                        lhsT=attnT_[0:bK, j, 32:64],
                        rhs=fnatB[0:bK, doff:doff + dw],
                        start=False, stop=True,
                        skip_group_check=True,
                        tile_position=(0, 32 * q))
                nc.vector.tensor_copy(out4[0:97, doff:doff + dw],
                                      cxp[0:97, 0:dw])
                if last:
                    # final group: store per chunk so the kernel-ending DMA
                    # only covers the last 256 columns (sync queue is idle)
                    nc.sync.dma_start(
                        ctx_out[b0:b0 + 4, doff:doff + dw],
                        out4.rearrange("(q r) d -> q r d", r=32)[:, 0,
                                                                 doff:doff + dw])
            if not last:
                nc.sync.dma_start(
                    ctx_out[b0:b0 + 4, :],
                    out4.rearrange("(q r) d -> q r d", r=32)[:, 0, :])

        for pi in range(NPAIR):
            if pi + 5 < NPAIR:
                loads(pi + 5)
            if pi == 0:
                head0_split()
                prolog_projh()
            else:
                head(pi)
            if pi % 2 == 0 and pi >= 2 and pi < NPAIR - 2:
                # softmax + ctx for the previous group: ACT ops issue
                # before tanh(pi) so the PE transposes/ctx never stall
                softmax_act(pi // 2 - 1)
                transp_flush(pi // 2 - 1)
            if pi == NPAIR - 1:
                # second-to-last group deferred to here: its ctx matmuls
                # keep the PE busy through tanh(15)'s ACT latency
                softmax_act(NPAIR // 2 - 2)
                transp_flush(NPAIR // 2 - 2)
            tanh_part(pi)
            logits(pi)
        softmax_act(NPAIR // 2 - 1)
        transp_flush(NPAIR // 2 - 1)


def _enable_jax_cache():
    try:
        import jax
        jax.config.update("jax_compilation_cache_dir", "/tmp/jax_neff_cache")
        jax.config.update("jax_persistent_cache_min_entry_size_bytes", 0)
        jax.config.update("jax_persistent_cache_min_compile_time_secs", 0)
    except Exception:
        pass


_enable_jax_cache()

_CACHE = {}


def _get_nc():
    if "nc" not in _CACHE:
        _CACHE["nc"] = build_kernel()
    return _CACHE["nc"]


def _run(inputs, trace=False):
    import ml_dtypes
    nc = _get_nc()
    feat_bf = np.ascontiguousarray(
        inputs["features"].astype(ml_dtypes.bfloat16))
    w1_bf = np.ascontiguousarray(inputs["W1"].astype(ml_dtypes.bfloat16))
    in_maps = []
    for c in range(NCORES):
        sl = slice(c * BS, (c + 1) * BS)
        in_maps.append({
            "features": feat_bf[sl],
            "hidden": np.ascontiguousarray(inputs["hidden"][sl]),
            "W1": w1_bf,
            "b1": np.ascontiguousarray(inputs["b1"]),
            "W2": np.ascontiguousarray(inputs["W2"]),
            "b2": np.ascontiguousarray(inputs["b2"]),
            "V": np.ascontiguousarray(inputs["V"]),
            "bV": np.ascontiguousarray(inputs["bV"]),
        })
    res = run_bass_kernel_spmd(nc, in_maps, core_ids=list(range(NCORES)),
                               trace=trace)
    out = np.concatenate([rr["context"] for rr in res.results], axis=0)
    return out, res


def kernel(**inputs):
    out, _ = _run(inputs, trace=False)
    return out


# revision 8
# speedup vs baseline: 1.0096x; 1.0026x over previous
"""Bahdanau attention kernel for Trainium2, 8-core data-parallel. v3.

Shapes (hardcoded): features [256,225,1280] f32, hidden [256,256] f32,
W1 [1280,256], b1 [256], W2 [256,256], b2 [256], V [256,1], bV [1].
Output: context [256,1280] f32.

Sharding: batch dim split across 8 cores (32 per core); parameters
replicated; no collectives.

Per-core pipeline (batch pairs, software-pipelined):
  - SWDGE cast-DMA loads features natural [L,D] as bf16, full-partition
    DMAs only (SWDGE 16-increment rule); last batch's chunk B loads
    shifted (rows l=97..224).
  - feature transposes run as fp32 transposes of bf16 PAIRS (2 PE
    cyc/row but half the columns; f32r would truncate the packed bits).
  - step-1 matmul reads featX with a stride-4B bf16 parity AP; W1 is
    DMA'd pre-permuted to the matching (slab, parity) row order.
  - scoreT[u,l] = tanh(W1.T @ featT + bias) on ScalarE with per-batch
    bias = proj_hT[:,b] + b1 + b2.
  - logits are batched per 2 pairs: a [128,2] V-stationary with V in
    column q2 accumulates pair q2's logits into row q2 of one PSUM
    bank; softmax (exp/recip/scale) then runs on [2,450] rows, and the
    four attn transposes move 2 columns each instead of 1.
  - context: groups of 4 batches in one PSUM bank at partitions
    0/32/64/96 via tile_position col-tiling (phase A then B), drained
    by one wide ScalarE copy. softmax+ctx for group g issue right
    after head(2g+2) so the PE never waits on ACT.
"""

import numpy as np

import concourse.bass as bass
import concourse.bacc as bacc
import concourse.tile as tile
import concourse.mybir as mybir
from concourse import masks
from concourse.bass_utils import run_bass_kernel_spmd

B, L, D, H, U = 256, 225, 1280, 256, 256
NCORES = 8
BS = B // NCORES          # 32 batch items per core
L0, L1 = 128, L - 128     # 128 + 97
NSLAB = D // 256          # 5 slabs of 256 d-values (128 fp32 pairs)
F32 = mybir.dt.float32
F32R = mybir.dt.float32r
BF16 = mybir.dt.bfloat16
AF = mybir.ActivationFunctionType


def build_kernel():
    nc = bacc.Bacc("TRN2", target_bir_lowering=False, debug=False, num_devices=NCORES)

    # features/W1 are pre-cast to bf16 on the host inside kernel() --
    # the kernel computes in bf16 anyway, so this halves the HBM read
    # without changing numerics.
    feat = nc.dram_tensor("features", [BS, L, D], BF16, kind="ExternalInput").ap()
    hid = nc.dram_tensor("hidden", [BS, H], F32, kind="ExternalInput").ap()
    w1 = nc.dram_tensor("W1", [D, U], BF16, kind="ExternalInput").ap()
    b1 = nc.dram_tensor("b1", [U], F32, kind="ExternalInput").ap()
    w2 = nc.dram_tensor("W2", [H, U], F32, kind="ExternalInput").ap()
    b2 = nc.dram_tensor("b2", [U], F32, kind="ExternalInput").ap()
    v = nc.dram_tensor("V", [U, 1], F32, kind="ExternalInput").ap()
    nc.dram_tensor("bV", [1], F32, kind="ExternalInput")  # softmax-invariant
    ctx_out = nc.dram_tensor("context", [BS, D], F32, kind="ExternalOutput").ap()

    with tile.TileContext(nc) as tc:
        body(tc, feat, hid, w1, b1, w2, b2, v, ctx_out)
    nc.compile()
    return nc


def body(tc, feat, hid, w1, b1, w2, b2, v, ctx_out):
    nc = tc.nc
    from contextlib import ExitStack

    with ExitStack() as ctx:
        const = ctx.enter_context(tc.tile_pool(name="const", bufs=1))
        fnat_pool = ctx.enter_context(tc.tile_pool(name="fnat", bufs=9))
        featX_pool = ctx.enter_context(tc.tile_pool(name="featX", bufs=3))
        score_pool = ctx.enter_context(tc.tile_pool(name="score", bufs=3))
        small = ctx.enter_context(tc.tile_pool(name="small", bufs=2))
        att_pool = ctx.enter_context(tc.tile_pool(name="att", bufs=2))
        outst_pool = ctx.enter_context(tc.tile_pool(name="outst", bufs=2))
        # PSUM bank budget (8): trp*3 + scp0 + scp1 + lgp*2(shared w/ atp)
        # + cxp*1 = 8
        pp = ctx.enter_context(tc.tile_pool(name="pp", bufs=1, space="PSUM"))

        NPAIR = BS // 2
        st = {}
        feat_flat = feat.rearrange("b l d -> (b l) d")

        # ---- identity first: the first transposes wait on it, and the
        # gpsimd queue behind it fills with DMA descriptor generation ----
        ident = const.tile([128, 128], F32)
        masks.make_identity(nc, ident[:, :])
        ident_b4 = const.tile([4, 4], BF16)  # bf16 identity for attn transposes
        nc.vector.tensor_copy(ident_b4[:, :], ident[0:4, 0:4])

        def loads(pi):
            # every DMA covers 128 partitions (see module docstring)
            fnats = []
            for half in range(2):
                b = 2 * pi + half
                fnat2 = fnat_pool.tile([128, 2, D], BF16, tag=f"fnat{half}",
                                       name=f"fnat_{pi}_{half}")
                if b < BS - 1:
                    # one DMA for both L-chunks: row p, chunk t reads
                    # feat[b, t*128+p, :] (t=1, p>=97 overreads into the
                    # next batch's rows -- harmless garbage, in bounds)
                    nc.gpsimd.dma_start(
                        fnat2[:, :, :],
                        feat_flat[b * L:b * L + 256, :].rearrange(
                            "(t p) d -> p t d", p=128))
                else:
                    # last batch: chunk B shifted, row p = l 97+p
                    nc.gpsimd.dma_start(fnat2[:, 0, :], feat[b, 0:128, :])
                    nc.gpsimd.dma_start(fnat2[:, 1, :],
                                        feat_flat[BS * L - 128:BS * L, :])
                fnats.append((fnat2[:, 0, :], fnat2[:, 1, :]))
            st[("fnats", pi)] = fnats

        def loads0():
            # pair 0 on the critical path: split fine so the first
            # transposes start as soon as the first slabs land
            fnats = []
            for half in range(2):
                b = half
                fnat2 = fnat_pool.tile([128, 2, D], BF16, tag=f"fnat{half}",
                                       name=f"fnat_0_{half}")
                if half == 0:
                    nc.gpsimd.dma_start(fnat2[:, 0, 0:512], feat[b, 0:128, 0:512])
                    nc.gpsimd.dma_start(fnat2[:, 0, 512:D], feat[b, 0:128, 512:D])
                    nc.gpsimd.dma_start(fnat2[:, 1, :],
                                        feat_flat[b * L + 128:b * L + 256, :])
                else:
                    nc.gpsimd.dma_start(
                        fnat2[:, :, :],
                        feat_flat[b * L:b * L + 256, :].rearrange(
                            "(t p) d -> p t d", p=128))
                fnats.append((fnat2[:, 0, :], fnat2[:, 1, :]))
            st[("fnats", 0)] = fnats

        loads0()

        ones32 = const.tile([1, 32], F32)
        nc.vector.memset(ones32[:, :], 1.0)
        ones32_r = const.tile([1, 32], BF16)
        nc.vector.tensor_copy(ones32_r[:, :], ones32[:, :])

        # ---- W1, pre-permuted to (slab, parity) rows matching featX:
        # partition p, (slab k, parity s) holds W1[256k + 2p + s, :] ----
        w1_sb = const.tile([128, NSLAB, 2, U], BF16)
        nc.scalar.dma_start(
            w1_sb[:, :, :, :],
            w1.rearrange("(k p s) u -> p k s u", p=128, s=2))

        loads(1)

        w2_sb = const.tile([128, 2, U], BF16)   # [h_in_tile, h_tile, u]
        nc.gpsimd.dma_start(w2_sb[:, :, :], w2.rearrange("(k p) u -> p k u", p=128))

        loads(2)

        # ---- V natural [1, 256]: one contiguous descriptor ----
        v_nat = const.tile([1, U], F32)
        nc.scalar.dma_start(v_nat[:, :], v.rearrange("u o -> o u"))

        bsum = const.tile([1, U], BF16)         # b1 + b2 (both added pre-tanh)
        b1_sb = const.tile([1, U], F32)
        b2_sb = const.tile([1, U], F32)
        nc.scalar.dma_start(b1_sb[:, :], b1[None, :])
        nc.scalar.dma_start(b2_sb[:, :], b2[None, :])
        nc.vector.tensor_add(bsum[:, :], b1_sb[:, :], b2_sb[:, :])

        hid_nat = const.tile([32, H], F32)
        nc.scalar.dma_start(hid_nat[:, :], hid[:, :])

        loads(3)
        loads(4)

        hidT = const.tile([128, 2, BS], BF16)   # [h_in_tile, h_tile, b]
        projhT = const.tile([128, 2 * BS], F32)  # [u_in_tile, ut*32+b]
        v_sb = const.tile([128, 2], BF16)       # [u_in_tile, u_tile]
        vq = const.tile([128, 2, 2, 2], BF16)   # [u_in, ut, q2, col]

        def prolog_projh():
            # proj_hT [u, b] = W2.T @ hiddenT + (b1+b2); emitted after
            # head(0) so the PE works on feature transposes while the
            # weight DMAs land. Also builds v_sb / vq from v_nat on-chip.
            vp = pp.tile([128, 512], F32, tag="trp", bufs=3, name="vp")
            for ut in range(2):
                nc.tensor.transpose(
                    vp[:, ut:ut + 1], v_nat[0:1, ut * 128:(ut + 1) * 128],
                    ident[0:1, 0:1])
            nc.vector.tensor_copy(v_sb[:, :], vp[:, 0:2])
            nc.vector.memset(vq[:, :, :, :], 0.0)
            for ut in range(2):
                for q2 in range(2):
                    nc.vector.tensor_copy(vq[:, ut, q2, q2:q2 + 1],
                                          v_sb[:, ut:ut + 1])
            for hk in range(2):
                hp = pp.tile([128, 512], F32, tag="trp", bufs=3)
                nc.tensor.transpose(
                    hp[:, 0:32], hid_nat[0:32, hk * 128:(hk + 1) * 128],
                    ident[0:32, 0:32])
                nc.vector.tensor_copy(hidT[:, hk, :], hp[:, 0:32])
            for ut in range(2):
                php = pp.tile([128, 512], F32, tag="trp", bufs=3)
                for hk in range(2):
                    nc.tensor.matmul(
                        php[:, 0:32],
                        lhsT=w2_sb[:, hk, ut * 128:(ut + 1) * 128],
                        rhs=hidT[:, hk, :],
                        start=(hk == 0), stop=False)
                nc.tensor.matmul(
                    php[:, 0:32],
                    lhsT=bsum[0:1, ut * 128:(ut + 1) * 128],
                    rhs=ones32_r[0:1, :].opt(),
                    start=False, stop=True)
                nc.vector.tensor_copy(projhT[:, ut * BS:(ut + 1) * BS], php[:, 0:32])

        # ---- main loop over batch pairs, software-pipelined ----

        def head0_split():
            # pair 0: process per batch so step-1 on batch 0 starts after
            # only half the pair's bytes (+W1) have landed
            fnats = st[("fnats", 0)]
            featX = featX_pool.tile([128, NSLAB, 2 * L], F32, tag="featX",
                                    name="featX_0")
            frs = [(fA.bitcast(F32), fB.bitcast(F32)) for fA, fB in fnats]
            scps = [pp.tile([128, 512], F32, tag=f"scp{ut}", bufs=1,
                            name=f"scp_0_{ut}") for ut in range(2)]
            featXb = featX[:, :, :].bitcast(BF16).rearrange(
                "p k (l s) -> p k l s", s=2)
            for half in range(2):
                fA32, fB32 = frs[half]
                for k in range(NSLAB):
                    trp = pp.tile([128, 452], F32, tag="trp", bufs=3,
                                  name=f"trp_0_{half}_{k}")
                    nc.tensor.transpose(trp[:, 0:128],
                                        fA32[:, k * 128:(k + 1) * 128],
                                        ident[:, :])
                    nc.tensor.transpose(trp[:, 128:128 + L1 + 1],
                                        fB32[0:L1 + 1, k * 128:(k + 1) * 128],
                                        ident[0:L1 + 1, 0:L1 + 1])
                    nc.vector.tensor_copy(featX[:, k, half * L:(half + 1) * L],
                                          trp[:, 0:L])
                for ut in range(2):
                    for k in range(NSLAB):
                        for s in range(2):
                            nc.tensor.matmul(
                                scps[ut][:, half * L:(half + 1) * L],
                                lhsT=w1_sb[:, k, s, ut * 128:(ut + 1) * 128],
                                rhs=featXb[:, k, half * L:(half + 1) * L, s],
                                start=(k == 0 and s == 0),
                                stop=(k == NSLAB - 1 and s == 1),
                                skip_group_check=True)
            st[("scps", 0)] = scps

        def head(pi):
            fnats = st[("fnats", pi)]
            # packed featT: fp32 lane p of slab k = d-pair (256k+2p, +1);
            # transposes run in f32r mode (1.5 PE cyc/row vs fp32 2.0)
            featX = featX_pool.tile([128, NSLAB, 2 * L], F32, tag="featX",
                                    name=f"featX_{pi}")
            frs = [(fA.bitcast(F32), fB.bitcast(F32)) for fA, fB in fnats]
            last = (pi == NPAIR - 1)
            for k in range(NSLAB):
                trp = pp.tile([128, 452], F32, tag="trp", bufs=3,
                              name=f"trp_{pi}_{k}")
                for half in range(2):
                    fA32, fB32 = frs[half]
                    o = 226 * half
                    nc.tensor.transpose(
                        trp[:, o:o + 128],
                        fA32[:, k * 128:(k + 1) * 128],
                        ident[:, :])
                    if last and half == 1:
                        # shifted chunk B: row p = l 97+p; cols o+97..o+224
                        # overlap chunk A's l 97..127 with identical values
                        nc.tensor.transpose(
                            trp[:, o + 97:o + 225],
                            fB32[:, k * 128:(k + 1) * 128],
                            ident[:, :])
                    else:
                        nc.tensor.transpose(
                            trp[:, o + 128:o + 128 + L1 + 1],
                            fB32[0:L1 + 1, k * 128:(k + 1) * 128],
                            ident[0:L1 + 1, 0:L1 + 1])
                nc.vector.tensor_copy(
                    featX[:, k, :],
                    trp.rearrange("p (h x) -> p h x", h=2)[:, :, 0:L])

            # bf16 parity view: [p, slab, l, s]
            featXb = featX[:, :, :].bitcast(BF16).rearrange(
                "p k (l s) -> p k l s", s=2)
            scps = []
            for ut in range(2):
                scp = pp.tile([128, 512], F32, tag=f"scp{ut}", bufs=1,
                              name=f"scp_{pi}_{ut}")
                for k in range(NSLAB):
                    for s in range(2):
                        nc.tensor.matmul(
                            scp[:, 0:2 * L],
                            lhsT=w1_sb[:, k, s, ut * 128:(ut + 1) * 128],
                            rhs=featXb[:, k, :, s],
                            start=(k == 0 and s == 0),
                            stop=(k == NSLAB - 1 and s == 1))
                scps.append(scp)
            st[("scps", pi)] = scps

        def tanh_part(pi):
            scps = st.pop(("scps", pi))
            score_sb = score_pool.tile([128, 2, 2 * L], BF16, tag="score_sb",
                                       name=f"score_{pi}")
            for ut in range(2):
                for half in range(2):
                    b = 2 * pi + half
                    nc.scalar.activation(
                        score_sb[:, ut, half * L:(half + 1) * L],
                        scps[ut][:, half * L:(half + 1) * L],
                        AF.Tanh,
                        bias=projhT[:, ut * BS + b:ut * BS + b + 1])
            st[("score", pi)] = score_sb

        def logits(pi):
            # rows 0/1 of one psum bank per 2-pair group, accumulated
            q2 = pi % 2
            if q2 == 0:
                lgp = pp.tile([128, 512], F32, tag="lgp", bufs=2,
                              name=f"lgp_{pi // 2}")
                st[("lgp", pi // 2)] = lgp
            else:
                lgp = st[("lgp", pi // 2)]
            score_sb = st.pop(("score", pi))
            for ut in range(2):
                nc.tensor.matmul(
                    lgp[0:2, 0:2 * L],
                    lhsT=vq[:, ut, q2, :],
                    rhs=score_sb[:, ut, :],
                    start=(q2 == 0 and ut == 0), stop=(q2 == 1 and ut == 1),
                    skip_group_check=True)

        def softmax_act(g):
            # exp/recip/scale for batches 4g..4g+3 on ACT+DVE ([2,450] rows)
            lgp = st.pop(("lgp", g))
            expl = small.tile([2, 2 * L], BF16, tag="expl", name=f"expl_{g}")
            esum = small.tile([2, 2], F32, tag="esum", name=f"esum_{g}")
            for half in range(2):
                nc.scalar.activation(
                    expl[0:2, half * L:(half + 1) * L],
                    lgp[0:2, half * L:(half + 1) * L], AF.Exp,
                    accum_out=esum[0:2, half:half + 1])
            rsum = small.tile([2, 2], F32, tag="rsum", name=f"rsum_{g}")
            nc.vector.reciprocal(rsum[:, :], esum[:, :])
            attn = small.tile([2, 2 * L], BF16, tag="attn", name=f"attn_{g}")
            for half in range(2):
                nc.vector.tensor_scalar_mul(
                    attn[0:2, half * L:(half + 1) * L],
                    expl[0:2, half * L:(half + 1) * L],
                    rsum[0:2, half:half + 1])
            st[("attn", g)] = attn

        def transp_flush(g):
            # attn transposes (2 cols per transpose) + ctx flush for the
            # 4 batches of group g
            attn = st.pop(("attn", g))
            last = (g == NPAIR // 2 - 1)
            # atp shares the lgp psum ring: same row bytes (bf16, 2x cols)
            atp = pp.tile([128, 1024], BF16, tag="lgp", bufs=2,
                          name=f"atp_{g}")
            nc.tensor.transpose(atp[0:128, 0:2], attn[0:2, 0:128],
                                ident_b4[0:2, 0:2])
            nc.tensor.transpose(atp[0:128, 2:4], attn[0:2, L:L + 128],
                                ident_b4[0:2, 0:2])
            nc.tensor.transpose(atp[0:97, 4:6], attn[0:2, 128:L],
                                ident_b4[0:2, 0:2])
            if last:
                # b31 (odd row 1) has shifted chunk B: stage a [2,128] row
                # pair -- row 0 (b29) at cols 0:97, row 1 (b31) shifted to
                # cols 31:128 -- then one 2-col transpose covers both.
                a31 = small.tile([2, 128], BF16, tag="a31", name="a31")
                nc.vector.memset(a31[:, :], 0.0)
                nc.vector.tensor_copy(a31[0:2, 31:128], attn[0:2, L + 128:2 * L])
                nc.vector.memset(a31[0:1, 97:128], 0.0)
                nc.vector.tensor_copy(a31[0:1, 0:97], attn[0:1, L + 128:2 * L])
                nc.tensor.transpose(atp[0:128, 6:8], a31[0:2, 0:128],
                                    ident_b4[0:2, 0:2])
            else:
                nc.tensor.transpose(atp[0:97, 6:8], attn[0:2, L + 128:2 * L],
                                    ident_b4[0:2, 0:2])

            attnT = att_pool.tile([128, 4, 64], BF16, tag="attnT",
                                  name=f"attnT_{g}")
            nc.vector.memset(attnT[:, :, :], 0.0)
            av = attnT.rearrange("p (j e) c -> p j e c", e=2)
            nc.vector.tensor_copy(av[:, :, 0, 0], atp[0:128, 0:2])
            nc.vector.tensor_copy(av[:, :, 1, 0], atp[0:128, 2:4])
            nc.vector.tensor_copy(av[0:97, :, 0, 32], atp[0:97, 4:6])
            if last:
                nc.vector.tensor_copy(av[:, :, 1, 32], atp[0:128, 6:8])
            else:
                nc.vector.tensor_copy(av[0:97, :, 1, 32], atp[0:97, 6:8])

            fn0 = st.pop(("fnats", 2 * g))
            fn1 = st.pop(("fnats", 2 * g + 1))
            group = []
            for j, (fnA, fnB) in enumerate(fn0 + fn1):
                bK = 128 if (last and j == 3) else L1
                group.append((4 * g + j, attnT, j, fnA, fnB, bK))

            # 4 batches -> one PSUM bank at partitions 0/32/64/96.
            out4 = outst_pool.tile([128, D], F32, tag="out_stage",
                                   name=f"outst_{group[0][0]}")
            b0 = group[0][0]
            for doff, dw in ((0, 512), (512, 512), (1024, 256)):
                cxp = pp.tile([128, 512], F32, tag="cxp", bufs=1,
                              name=f"cxp_{group[0][0]}_{doff}")
                for q, (b, attnT_, j, fnatA, fnatB, bK) in enumerate(group):
                    nc.tensor.matmul(
                        cxp[32 * q:32 * q + 32, 0:dw],
                        lhsT=attnT_[0:128, j, 0:32],
                        rhs=fnatA[:, doff:doff + dw],
                        start=True, stop=False,
                        skip_group_check=True,
                        tile_position=(0, 32 * q))
                for q, (b, attnT_, j, fnatA, fnatB, bK) in enumerate(group):
                    nc.tensor.matmul(
                        cxp[32 * q:32 * q + 32, 0:dw],
                        lhsT=attnT_[0:bK, j, 32:64],
                        rhs=fnatB[0:bK, doff:doff + dw],
                        start=False, stop=True,
                        skip_group_check=True,
                        tile_position=(0, 32 * q))
                nc.scalar.copy(out4[0:97, doff:doff + dw],
                               cxp[0:97, 0:dw])
                if last:
                    # final group: store per chunk so the kernel-ending DMA
                    # only covers the last 256 columns (sync queue is idle)
                    nc.sync.dma_start(
                        ctx_out[b0:b0 + 4, doff:doff + dw],
                        out4.rearrange("(q r) d -> q r d", r=32)[:, 0,
                                                                 doff:doff + dw])
            if not last:
                nc.sync.dma_start(
                    ctx_out[b0:b0 + 4, :],
                    out4.rearrange("(q r) d -> q r d", r=32)[:, 0, :])

        for pi in range(NPAIR):
            if pi + 5 < NPAIR:
                loads(pi + 5)
            if pi == 0:
                head0_split()
                prolog_projh()
            else:
                head(pi)
            if pi % 2 == 0 and pi >= 2 and pi < NPAIR - 2:
                # softmax + ctx for the previous group: ACT ops issue
                # before tanh(pi) so the PE transposes/ctx never stall
                softmax_act(pi // 2 - 1)
                transp_flush(pi // 2 - 1)
            if pi == NPAIR - 1:
                # second-to-last group deferred to here: its ctx matmuls
                # keep the PE busy through tanh(15)'s ACT latency
                softmax_act(NPAIR // 2 - 2)
                transp_flush(NPAIR // 2 - 2)
            tanh_part(pi)
            logits(pi)
        softmax_act(NPAIR // 2 - 1)
        transp_flush(NPAIR // 2 - 1)


def _enable_jax_cache():
    try:
        import jax
        jax.config.update("jax_compilation_cache_dir", "/tmp/jax_neff_cache")
        jax.config.update("jax_persistent_cache_min_entry_size_bytes", 0)
        jax.config.update("jax_persistent_cache_min_compile_time_secs", 0)
    except Exception:
        pass


_enable_jax_cache()

_CACHE = {}


def _get_nc():
    if "nc" not in _CACHE:
        _CACHE["nc"] = build_kernel()
    return _CACHE["nc"]


def _run(inputs, trace=False):
    import ml_dtypes
    nc = _get_nc()
    # np.asarray first: harness may pass jax arrays, and jax .astype under
    # the axon platform would trigger a device JIT -- convert on host.
    feat_bf = np.ascontiguousarray(
        np.asarray(inputs["features"]).astype(ml_dtypes.bfloat16))
    w1_bf = np.ascontiguousarray(
        np.asarray(inputs["W1"]).astype(ml_dtypes.bfloat16))
    in_maps = []
    for c in range(NCORES):
        sl = slice(c * BS, (c + 1) * BS)
        in_maps.append({
            "features": feat_bf[sl],
            "hidden": np.ascontiguousarray(np.asarray(inputs["hidden"])[sl]),
            "W1": w1_bf,
            "b1": np.ascontiguousarray(np.asarray(inputs["b1"])),
            "W2": np.ascontiguousarray(np.asarray(inputs["W2"])),
            "b2": np.ascontiguousarray(np.asarray(inputs["b2"])),
            "V": np.ascontiguousarray(np.asarray(inputs["V"])),
            "bV": np.ascontiguousarray(np.asarray(inputs["bV"])),
        })
    res = run_bass_kernel_spmd(nc, in_maps, core_ids=list(range(NCORES)),
                               trace=trace)
    out = np.concatenate([rr["context"] for rr in res.results], axis=0)
    return out, res


def kernel(**inputs):
    out, _ = _run(inputs, trace=False)
    return out


# revision 9
# speedup vs baseline: 1.0117x; 1.0021x over previous
"""Bahdanau attention kernel for Trainium2, 8-core data-parallel. v3.

Shapes (hardcoded): features [256,225,1280] f32, hidden [256,256] f32,
W1 [1280,256], b1 [256], W2 [256,256], b2 [256], V [256,1], bV [1].
Output: context [256,1280] f32.

Sharding: batch dim split across 8 cores (32 per core); parameters
replicated; no collectives.

Per-core pipeline (batch pairs, software-pipelined):
  - SWDGE cast-DMA loads features natural [L,D] as bf16, full-partition
    DMAs only (SWDGE 16-increment rule); last batch's chunk B loads
    shifted (rows l=97..224).
  - feature transposes run as fp32 transposes of bf16 PAIRS (2 PE
    cyc/row but half the columns; f32r would truncate the packed bits).
  - step-1 matmul reads featX with a stride-4B bf16 parity AP; W1 is
    DMA'd pre-permuted to the matching (slab, parity) row order.
  - scoreT[u,l] = tanh(W1.T @ featT + bias) on ScalarE with per-batch
    bias = proj_hT[:,b] + b1 + b2.
  - logits are batched per 2 pairs: a [128,2] V-stationary with V in
    column q2 accumulates pair q2's logits into row q2 of one PSUM
    bank; softmax (exp/recip/scale) then runs on [2,450] rows, and the
    four attn transposes move 2 columns each instead of 1.
  - context: groups of 4 batches in one PSUM bank at partitions
    0/32/64/96 via tile_position col-tiling (phase A then B), drained
    by one wide ScalarE copy. softmax+ctx for group g issue right
    after head(2g+2) so the PE never waits on ACT.
"""

import numpy as np

import concourse.bass as bass
import concourse.bacc as bacc
import concourse.tile as tile
import concourse.mybir as mybir
from concourse import masks
from concourse.bass_utils import run_bass_kernel_spmd

B, L, D, H, U = 256, 225, 1280, 256, 256
NCORES = 8
BS = B // NCORES          # 32 batch items per core
L0, L1 = 128, L - 128     # 128 + 97
NSLAB = D // 256          # 5 slabs of 256 d-values (128 fp32 pairs)
F32 = mybir.dt.float32
F32R = mybir.dt.float32r
BF16 = mybir.dt.bfloat16
AF = mybir.ActivationFunctionType


def build_kernel():
    nc = bacc.Bacc("TRN2", target_bir_lowering=False, debug=False, num_devices=NCORES)

    # features/W1 are pre-cast to bf16 on the host inside kernel() --
    # the kernel computes in bf16 anyway, so this halves the HBM read
    # without changing numerics.
    feat = nc.dram_tensor("features", [BS, L, D], BF16, kind="ExternalInput").ap()
    hid = nc.dram_tensor("hidden", [BS, H], F32, kind="ExternalInput").ap()
    w1 = nc.dram_tensor("W1", [D, U], BF16, kind="ExternalInput").ap()
    b1 = nc.dram_tensor("b1", [U], F32, kind="ExternalInput").ap()
    w2 = nc.dram_tensor("W2", [H, U], F32, kind="ExternalInput").ap()
    b2 = nc.dram_tensor("b2", [U], F32, kind="ExternalInput").ap()
    v = nc.dram_tensor("V", [U, 1], F32, kind="ExternalInput").ap()
    nc.dram_tensor("bV", [1], F32, kind="ExternalInput")  # softmax-invariant
    ctx_out = nc.dram_tensor("context", [BS, D], F32, kind="ExternalOutput").ap()

    with tile.TileContext(nc) as tc:
        body(tc, feat, hid, w1, b1, w2, b2, v, ctx_out)
    nc.compile()
    return nc


def body(tc, feat, hid, w1, b1, w2, b2, v, ctx_out):
    nc = tc.nc
    from contextlib import ExitStack

    with ExitStack() as ctx:
        const = ctx.enter_context(tc.tile_pool(name="const", bufs=1))
        fnat_pool = ctx.enter_context(tc.tile_pool(name="fnat", bufs=9))
        featX_pool = ctx.enter_context(tc.tile_pool(name="featX", bufs=3))
        score_pool = ctx.enter_context(tc.tile_pool(name="score", bufs=3))
        small = ctx.enter_context(tc.tile_pool(name="small", bufs=2))
        att_pool = ctx.enter_context(tc.tile_pool(name="att", bufs=2))
        outst_pool = ctx.enter_context(tc.tile_pool(name="outst", bufs=2))
        # PSUM bank budget (8): trp*3 + scp0 + scp1 + lgp*2(shared w/ atp)
        # + cxp*1 = 8
        pp = ctx.enter_context(tc.tile_pool(name="pp", bufs=1, space="PSUM"))

        NPAIR = BS // 2
        st = {}
        feat_flat = feat.rearrange("b l d -> (b l) d")

        # ---- identity first: the first transposes wait on it, and the
        # gpsimd queue behind it fills with DMA descriptor generation ----
        ident = const.tile([128, 128], F32)
        masks.make_identity(nc, ident[:, :])
        ident_b4 = const.tile([4, 4], BF16)  # bf16 identity for attn transposes
        nc.vector.tensor_copy(ident_b4[:, :], ident[0:4, 0:4])

        def loads(pi):
            # every DMA covers 128 partitions (see module docstring)
            fnats = []
            for half in range(2):
                b = 2 * pi + half
                fnat2 = fnat_pool.tile([128, 2, D], BF16, tag=f"fnat{half}",
                                       name=f"fnat_{pi}_{half}")
                if b < BS - 1:
                    # one DMA for both L-chunks: row p, chunk t reads
                    # feat[b, t*128+p, :] (t=1, p>=97 overreads into the
                    # next batch's rows -- harmless garbage, in bounds)
                    nc.gpsimd.dma_start(
                        fnat2[:, :, :],
                        feat_flat[b * L:b * L + 256, :].rearrange(
                            "(t p) d -> p t d", p=128))
                else:
                    # last batch: chunk B shifted, row p = l 97+p
                    nc.gpsimd.dma_start(fnat2[:, 0, :], feat[b, 0:128, :])
                    nc.gpsimd.dma_start(fnat2[:, 1, :],
                                        feat_flat[BS * L - 128:BS * L, :])
                fnats.append((fnat2[:, 0, :], fnat2[:, 1, :]))
            st[("fnats", pi)] = fnats

        def loads0():
            # pair 0 on the critical path: split fine so the first
            # transposes start as soon as the first slabs land
            fnats = []
            for half in range(2):
                b = half
                fnat2 = fnat_pool.tile([128, 2, D], BF16, tag=f"fnat{half}",
                                       name=f"fnat_0_{half}")
                if half == 0:
                    nc.gpsimd.dma_start(fnat2[:, 0, 0:512], feat[b, 0:128, 0:512])
                    nc.gpsimd.dma_start(fnat2[:, 0, 512:D], feat[b, 0:128, 512:D])
                    nc.gpsimd.dma_start(fnat2[:, 1, :],
                                        feat_flat[b * L + 128:b * L + 256, :])
                else:
                    nc.gpsimd.dma_start(
                        fnat2[:, :, :],
                        feat_flat[b * L:b * L + 256, :].rearrange(
                            "(t p) d -> p t d", p=128))
                fnats.append((fnat2[:, 0, :], fnat2[:, 1, :]))
            st[("fnats", 0)] = fnats

        loads0()

        ones32 = const.tile([1, 32], F32)
        nc.vector.memset(ones32[:, :], 1.0)
        ones32_r = const.tile([1, 32], BF16)
        nc.vector.tensor_copy(ones32_r[:, :], ones32[:, :])

        # ---- W1, pre-permuted to (slab, parity) rows matching featX:
        # partition p, (slab k, parity s) holds W1[256k + 2p + s, :] ----
        w1_sb = const.tile([128, NSLAB, 2, U], BF16)
        nc.scalar.dma_start(
            w1_sb[:, :, :, :],
            w1.rearrange("(k p s) u -> p k s u", p=128, s=2))

        loads(1)

        w2_sb = const.tile([128, 2, U], BF16)   # [h_in_tile, h_tile, u]
        nc.gpsimd.dma_start(w2_sb[:, :, :], w2.rearrange("(k p) u -> p k u", p=128))

        loads(2)

        # ---- V natural [1, 256]: one contiguous descriptor ----
        v_nat = const.tile([1, U], F32)
        nc.scalar.dma_start(v_nat[:, :], v.rearrange("u o -> o u"))

        bsum = const.tile([1, U], BF16)         # b1 + b2 (both added pre-tanh)
        b1_sb = const.tile([1, U], F32)
        b2_sb = const.tile([1, U], F32)
        nc.scalar.dma_start(b1_sb[:, :], b1[None, :])
        nc.scalar.dma_start(b2_sb[:, :], b2[None, :])
        nc.vector.tensor_add(bsum[:, :], b1_sb[:, :], b2_sb[:, :])

        hid_nat = const.tile([32, H], F32)
        nc.scalar.dma_start(hid_nat[:, :], hid[:, :])

        loads(3)
        loads(4)

        hidT = const.tile([128, 2, BS], BF16)   # [h_in_tile, h_tile, b]
        projhT = const.tile([128, 2 * BS], F32)  # [u_in_tile, ut*32+b]
        v_sb = const.tile([128, 2], BF16)       # [u_in_tile, u_tile]
        vq = const.tile([128, 2, 2, 2], BF16)   # [u_in, ut, q2, col]

        def prolog_projh():
            # proj_hT [u, b] = W2.T @ hiddenT + (b1+b2); emitted after
            # head(0) so the PE works on feature transposes while the
            # weight DMAs land. Also builds v_sb / vq from v_nat on-chip.
            vp = pp.tile([128, 512], F32, tag="trp", bufs=3, name="vp")
            for ut in range(2):
                nc.tensor.transpose(
                    vp[:, ut:ut + 1], v_nat[0:1, ut * 128:(ut + 1) * 128],
                    ident[0:1, 0:1])
            nc.vector.tensor_copy(v_sb[:, :], vp[:, 0:2])
            nc.vector.memset(vq[:, :, :, :], 0.0)
            for ut in range(2):
                for q2 in range(2):
                    nc.vector.tensor_copy(vq[:, ut, q2, q2:q2 + 1],
                                          v_sb[:, ut:ut + 1])
            for hk in range(2):
                hp = pp.tile([128, 512], F32, tag="trp", bufs=3)
                nc.tensor.transpose(
                    hp[:, 0:32], hid_nat[0:32, hk * 128:(hk + 1) * 128],
                    ident[0:32, 0:32])
                nc.vector.tensor_copy(hidT[:, hk, :], hp[:, 0:32])
            for ut in range(2):
                php = pp.tile([128, 512], F32, tag="trp", bufs=3)
                for hk in range(2):
                    nc.tensor.matmul(
                        php[:, 0:32],
                        lhsT=w2_sb[:, hk, ut * 128:(ut + 1) * 128],
                        rhs=hidT[:, hk, :],
                        start=(hk == 0), stop=False)
                nc.tensor.matmul(
                    php[:, 0:32],
                    lhsT=bsum[0:1, ut * 128:(ut + 1) * 128],
                    rhs=ones32_r[0:1, :].opt(),
                    start=False, stop=True)
                nc.vector.tensor_copy(projhT[:, ut * BS:(ut + 1) * BS], php[:, 0:32])

        # ---- main loop over batch pairs, software-pipelined ----

        def head0_split():
            # pair 0: process per batch so step-1 on batch 0 starts after
            # only half the pair's bytes (+W1) have landed
            fnats = st[("fnats", 0)]
            featX = featX_pool.tile([128, NSLAB, 2 * L], F32, tag="featX",
                                    name="featX_0")
            frs = [(fA.bitcast(F32), fB.bitcast(F32)) for fA, fB in fnats]
            scps = [pp.tile([128, 512], F32, tag=f"scp{ut}", bufs=1,
                            name=f"scp_0_{ut}") for ut in range(2)]
            featXb = featX[:, :, :].bitcast(BF16).rearrange(
                "p k (l s) -> p k l s", s=2)
            for half in range(2):
                fA32, fB32 = frs[half]
                for k in range(NSLAB):
                    trp = pp.tile([128, 452], F32, tag="trp", bufs=3,
                                  name=f"trp_0_{half}_{k}")
                    nc.tensor.transpose(trp[:, 0:128],
                                        fA32[:, k * 128:(k + 1) * 128],
                                        ident[:, :])
                    nc.tensor.transpose(trp[:, 128:128 + L1 + 1],
                                        fB32[0:L1 + 1, k * 128:(k + 1) * 128],
                                        ident[0:L1 + 1, 0:L1 + 1])
                    nc.vector.tensor_copy(featX[:, k, half * L:(half + 1) * L],
                                          trp[:, 0:L])
                for ut in range(2):
                    for k in range(NSLAB):
                        for s in range(2):
                            nc.tensor.matmul(
                                scps[ut][:, half * L:(half + 1) * L],
                                lhsT=w1_sb[:, k, s, ut * 128:(ut + 1) * 128],
                                rhs=featXb[:, k, half * L:(half + 1) * L, s],
                                start=(k == 0 and s == 0),
                                stop=(k == NSLAB - 1 and s == 1),
                                skip_group_check=True)
            st[("scps", 0)] = scps

        def head(pi):
            fnats = st[("fnats", pi)]
            # packed featT: fp32 lane p of slab k = d-pair (256k+2p, +1);
            # transposes run in f32r mode (1.5 PE cyc/row vs fp32 2.0)
            featX = featX_pool.tile([128, NSLAB, 2 * L], F32, tag="featX",
                                    name=f"featX_{pi}")
            frs = [(fA.bitcast(F32), fB.bitcast(F32)) for fA, fB in fnats]
            last = (pi == NPAIR - 1)
            for k in range(NSLAB):
                trp = pp.tile([128, 452], F32, tag="trp", bufs=3,
                              name=f"trp_{pi}_{k}")
                for half in range(2):
                    fA32, fB32 = frs[half]
                    o = 226 * half
                    nc.tensor.transpose(
                        trp[:, o:o + 128],
                        fA32[:, k * 128:(k + 1) * 128],
                        ident[:, :])
                    if last and half == 1:
                        # shifted chunk B: row p = l 97+p; cols o+97..o+224
                        # overlap chunk A's l 97..127 with identical values
                        nc.tensor.transpose(
                            trp[:, o + 97:o + 225],
                            fB32[:, k * 128:(k + 1) * 128],
                            ident[:, :])
                    else:
                        nc.tensor.transpose(
                            trp[:, o + 128:o + 128 + L1 + 1],
                            fB32[0:L1 + 1, k * 128:(k + 1) * 128],
                            ident[0:L1 + 1, 0:L1 + 1])
                nc.vector.tensor_copy(
                    featX[:, k, :],
                    trp.rearrange("p (h x) -> p h x", h=2)[:, :, 0:L])

            # bf16 parity view: [p, slab, l, s]
            featXb = featX[:, :, :].bitcast(BF16).rearrange(
                "p k (l s) -> p k l s", s=2)
            scps = []
            for ut in range(2):
                scp = pp.tile([128, 512], F32, tag=f"scp{ut}", bufs=1,
                              name=f"scp_{pi}_{ut}")
                for k in range(NSLAB):
                    for s in range(2):
                        nc.tensor.matmul(
                            scp[:, 0:2 * L],
                            lhsT=w1_sb[:, k, s, ut * 128:(ut + 1) * 128],
                            rhs=featXb[:, k, :, s],
                            start=(k == 0 and s == 0),
                            stop=(k == NSLAB - 1 and s == 1))
                scps.append(scp)
            st[("scps", pi)] = scps

        def tanh_part(pi):
            scps = st.pop(("scps", pi))
            score_sb = score_pool.tile([128, 2, 2 * L], BF16, tag="score_sb",
                                       name=f"score_{pi}")
            for ut in range(2):
                for half in range(2):
                    b = 2 * pi + half
                    nc.scalar.activation(
                        score_sb[:, ut, half * L:(half + 1) * L],
                        scps[ut][:, half * L:(half + 1) * L],
                        AF.Tanh,
                        bias=projhT[:, ut * BS + b:ut * BS + b + 1])
            st[("score", pi)] = score_sb

        def logits(pi):
            # rows 0/1 of one psum bank per 2-pair group, accumulated
            q2 = pi % 2
            if q2 == 0:
                lgp = pp.tile([128, 512], F32, tag="lgp", bufs=2,
                              name=f"lgp_{pi // 2}")
                st[("lgp", pi // 2)] = lgp
            else:
                lgp = st[("lgp", pi // 2)]
            score_sb = st.pop(("score", pi))
            for ut in range(2):
                nc.tensor.matmul(
                    lgp[0:2, 0:2 * L],
                    lhsT=vq[:, ut, q2, :],
                    rhs=score_sb[:, ut, :],
                    start=(q2 == 0 and ut == 0), stop=(q2 == 1 and ut == 1),
                    skip_group_check=True)

        def softmax_act(g):
            # exp/recip/scale for batches 4g..4g+3 on ACT+DVE ([2,450] rows)
            lgp = st.pop(("lgp", g))
            expl = small.tile([2, 2 * L], BF16, tag="expl", name=f"expl_{g}")
            esum = small.tile([2, 2], F32, tag="esum", name=f"esum_{g}")
            for half in range(2):
                nc.scalar.activation(
                    expl[0:2, half * L:(half + 1) * L],
                    lgp[0:2, half * L:(half + 1) * L], AF.Exp,
                    accum_out=esum[0:2, half:half + 1])
            rsum = small.tile([2, 2], F32, tag="rsum", name=f"rsum_{g}")
            nc.vector.reciprocal(rsum[:, :], esum[:, :])
            attn = small.tile([2, 2 * L], BF16, tag="attn", name=f"attn_{g}")
            for half in range(2):
                nc.vector.tensor_scalar_mul(
                    attn[0:2, half * L:(half + 1) * L],
                    expl[0:2, half * L:(half + 1) * L],
                    rsum[0:2, half:half + 1])
            st[("attn", g)] = attn

        def transp_flush(g):
            # attn transposes (2 cols per transpose) + ctx flush for the
            # 4 batches of group g
            attn = st.pop(("attn", g))
            last = (g == NPAIR // 2 - 1)
            # atp shares the lgp psum ring: same row bytes (bf16, 2x cols)
            atp = pp.tile([128, 1024], BF16, tag="lgp", bufs=2,
                          name=f"atp_{g}")
            nc.tensor.transpose(atp[0:128, 0:2], attn[0:2, 0:128],
                                ident_b4[0:2, 0:2])
            nc.tensor.transpose(atp[0:128, 2:4], attn[0:2, L:L + 128],
                                ident_b4[0:2, 0:2])
            nc.tensor.transpose(atp[0:97, 4:6], attn[0:2, 128:L],
                                ident_b4[0:2, 0:2])
            if last:
                # b31 (odd row 1) has shifted chunk B: stage a [2,128] row
                # pair -- row 0 (b29) at cols 0:97, row 1 (b31) shifted to
                # cols 31:128 -- then one 2-col transpose covers both.
                a31 = small.tile([2, 128], BF16, tag="a31", name="a31")
                nc.vector.memset(a31[:, :], 0.0)
                nc.vector.tensor_copy(a31[0:2, 31:128], attn[0:2, L + 128:2 * L])
                nc.vector.memset(a31[0:1, 97:128], 0.0)
                nc.vector.tensor_copy(a31[0:1, 0:97], attn[0:1, L + 128:2 * L])
                nc.tensor.transpose(atp[0:128, 6:8], a31[0:2, 0:128],
                                    ident_b4[0:2, 0:2])
            else:
                nc.tensor.transpose(atp[0:97, 6:8], attn[0:2, L + 128:2 * L],
                                    ident_b4[0:2, 0:2])

            attnT = att_pool.tile([128, 4, 64], BF16, tag="attnT",
                                  name=f"attnT_{g}")
            if g < 2:
                # zeros persist across ring reuses: the copies below only
                # ever write cols 0/32 of each 64-col group (rows 0:97 for
                # chunk B), so cells zeroed here are never dirtied
                nc.vector.memset(attnT[:, :, :], 0.0)
            av = attnT.rearrange("p (j e) c -> p j e c", e=2)
            nc.vector.tensor_copy(av[:, :, 0, 0], atp[0:128, 0:2])
            nc.vector.tensor_copy(av[:, :, 1, 0], atp[0:128, 2:4])
            nc.vector.tensor_copy(av[0:97, :, 0, 32], atp[0:97, 4:6])
            if last:
                nc.vector.tensor_copy(av[:, :, 1, 32], atp[0:128, 6:8])
            else:
                nc.vector.tensor_copy(av[0:97, :, 1, 32], atp[0:97, 6:8])

            fn0 = st.pop(("fnats", 2 * g))
            fn1 = st.pop(("fnats", 2 * g + 1))
            group = []
            for j, (fnA, fnB) in enumerate(fn0 + fn1):
                bK = 128 if (last and j == 3) else L1
                group.append((4 * g + j, attnT, j, fnA, fnB, bK))

            # 4 batches -> one PSUM bank at partitions 0/32/64/96.
            out4 = outst_pool.tile([128, D], F32, tag="out_stage",
                                   name=f"outst_{group[0][0]}")
            b0 = group[0][0]
            for doff, dw in ((0, 512), (512, 512), (1024, 256)):
                cxp = pp.tile([128, 512], F32, tag="cxp", bufs=1,
                              name=f"cxp_{group[0][0]}_{doff}")
                for q, (b, attnT_, j, fnatA, fnatB, bK) in enumerate(group):
                    nc.tensor.matmul(
                        cxp[32 * q:32 * q + 32, 0:dw],
                        lhsT=attnT_[0:128, j, 0:32],
                        rhs=fnatA[:, doff:doff + dw],
                        start=True, stop=False,
                        skip_group_check=True,
                        tile_position=(0, 32 * q))
                for q, (b, attnT_, j, fnatA, fnatB, bK) in enumerate(group):
                    nc.tensor.matmul(
                        cxp[32 * q:32 * q + 32, 0:dw],
                        lhsT=attnT_[0:bK, j, 32:64],
                        rhs=fnatB[0:bK, doff:doff + dw],
                        start=False, stop=True,
                        skip_group_check=True,
                        tile_position=(0, 32 * q))
                nc.scalar.copy(out4[0:97, doff:doff + dw],
                               cxp[0:97, 0:dw])
                if last:
                    # final group: store per chunk so the kernel-ending DMA
                    # only covers the last 256 columns (sync queue is idle)
                    nc.sync.dma_start(
                        ctx_out[b0:b0 + 4, doff:doff + dw],
                        out4.rearrange("(q r) d -> q r d", r=32)[:, 0,
                                                                 doff:doff + dw])
            if not last:
                nc.sync.dma_start(
                    ctx_out[b0:b0 + 4, :],
                    out4.rearrange("(q r) d -> q r d", r=32)[:, 0, :])

        for pi in range(NPAIR):
            if pi + 5 < NPAIR:
                loads(pi + 5)
            if pi == 0:
                head0_split()
                prolog_projh()
            else:
                head(pi)
            if pi % 2 == 0 and pi >= 2 and pi < NPAIR - 2:
                # softmax + ctx for the previous group: ACT ops issue
                # before tanh(pi) so the PE transposes/ctx never stall
                softmax_act(pi // 2 - 1)
                transp_flush(pi // 2 - 1)
            if pi == NPAIR - 1:
                # second-to-last group deferred to here: its ctx matmuls
                # keep the PE busy through tanh(15)'s ACT latency
                softmax_act(NPAIR // 2 - 2)
                transp_flush(NPAIR // 2 - 2)
            tanh_part(pi)
            logits(pi)
        softmax_act(NPAIR // 2 - 1)
        transp_flush(NPAIR // 2 - 1)


def _enable_jax_cache():
    try:
        import jax
        jax.config.update("jax_compilation_cache_dir", "/tmp/jax_neff_cache")
        jax.config.update("jax_persistent_cache_min_entry_size_bytes", 0)
        jax.config.update("jax_persistent_cache_min_compile_time_secs", 0)
    except Exception:
        pass


_enable_jax_cache()

_CACHE = {}


def _get_nc():
    if "nc" not in _CACHE:
        _CACHE["nc"] = build_kernel()
    return _CACHE["nc"]


def _run(inputs, trace=False):
    import ml_dtypes
    nc = _get_nc()
    # np.asarray first: harness may pass jax arrays, and jax .astype under
    # the axon platform would trigger a device JIT -- convert on host.
    feat_bf = np.ascontiguousarray(
        np.asarray(inputs["features"]).astype(ml_dtypes.bfloat16))
    w1_bf = np.ascontiguousarray(
        np.asarray(inputs["W1"]).astype(ml_dtypes.bfloat16))
    in_maps = []
    for c in range(NCORES):
        sl = slice(c * BS, (c + 1) * BS)
        in_maps.append({
            "features": feat_bf[sl],
            "hidden": np.ascontiguousarray(np.asarray(inputs["hidden"])[sl]),
            "W1": w1_bf,
            "b1": np.ascontiguousarray(np.asarray(inputs["b1"])),
            "W2": np.ascontiguousarray(np.asarray(inputs["W2"])),
            "b2": np.ascontiguousarray(np.asarray(inputs["b2"])),
            "V": np.ascontiguousarray(np.asarray(inputs["V"])),
            "bV": np.ascontiguousarray(np.asarray(inputs["bV"])),
        })
    res = run_bass_kernel_spmd(nc, in_maps, core_ids=list(range(NCORES)),
                               trace=trace)
    out = np.concatenate([rr["context"] for rr in res.results], axis=0)
    return out, res


def kernel(**inputs):
    out, _ = _run(inputs, trace=False)
    return out


# revision 10
# speedup vs baseline: 1.0129x; 1.0012x over previous
"""Bahdanau attention kernel for Trainium2, 8-core data-parallel. v3.

Shapes (hardcoded): features [256,225,1280] f32, hidden [256,256] f32,
W1 [1280,256], b1 [256], W2 [256,256], b2 [256], V [256,1], bV [1].
Output: context [256,1280] f32.

Sharding: batch dim split across 8 cores (32 per core); parameters
replicated; no collectives.

Per-core pipeline (batch pairs, software-pipelined):
  - SWDGE cast-DMA loads features natural [L,D] as bf16, full-partition
    DMAs only (SWDGE 16-increment rule); last batch's chunk B loads
    shifted (rows l=97..224).
  - feature transposes run as fp32 transposes of bf16 PAIRS (2 PE
    cyc/row but half the columns; f32r would truncate the packed bits).
  - step-1 matmul reads featX with a stride-4B bf16 parity AP; W1 is
    DMA'd pre-permuted to the matching (slab, parity) row order.
  - scoreT[u,l] = tanh(W1.T @ featT + bias) on ScalarE with per-batch
    bias = proj_hT[:,b] + b1 + b2.
  - logits are batched per 2 pairs: a [128,2] V-stationary with V in
    column q2 accumulates pair q2's logits into row q2 of one PSUM
    bank; softmax (exp/recip/scale) then runs on [2,450] rows, and the
    four attn transposes move 2 columns each instead of 1.
  - context: groups of 4 batches in one PSUM bank at partitions
    0/32/64/96 via tile_position col-tiling (phase A then B), drained
    by one wide ScalarE copy. softmax+ctx for group g issue right
    after head(2g+2) so the PE never waits on ACT.
"""

import numpy as np

import concourse.bass as bass
import concourse.bacc as bacc
import concourse.tile as tile
import concourse.mybir as mybir
from concourse import masks
from concourse.bass_utils import run_bass_kernel_spmd

B, L, D, H, U = 256, 225, 1280, 256, 256
NCORES = 8
BS = B // NCORES          # 32 batch items per core
L0, L1 = 128, L - 128     # 128 + 97
NSLAB = D // 256          # 5 slabs of 256 d-values (128 fp32 pairs)
F32 = mybir.dt.float32
F32R = mybir.dt.float32r
BF16 = mybir.dt.bfloat16
AF = mybir.ActivationFunctionType


def build_kernel():
    nc = bacc.Bacc("TRN2", target_bir_lowering=False, debug=False, num_devices=NCORES)

    # features/W1 are pre-cast to bf16 on the host inside kernel() --
    # the kernel computes in bf16 anyway, so this halves the HBM read
    # without changing numerics.
    feat = nc.dram_tensor("features", [BS, L, D], BF16, kind="ExternalInput").ap()
    hid = nc.dram_tensor("hidden", [BS, H], F32, kind="ExternalInput").ap()
    w1 = nc.dram_tensor("W1", [D, U], BF16, kind="ExternalInput").ap()
    b1 = nc.dram_tensor("b1", [U], F32, kind="ExternalInput").ap()
    w2 = nc.dram_tensor("W2", [H, U], BF16, kind="ExternalInput").ap()
    b2 = nc.dram_tensor("b2", [U], F32, kind="ExternalInput").ap()
    v = nc.dram_tensor("V", [U, 1], F32, kind="ExternalInput").ap()
    nc.dram_tensor("bV", [1], F32, kind="ExternalInput")  # softmax-invariant
    ctx_out = nc.dram_tensor("context", [BS, D], F32, kind="ExternalOutput").ap()

    with tile.TileContext(nc) as tc:
        body(tc, feat, hid, w1, b1, w2, b2, v, ctx_out)
    nc.compile()
    return nc


def body(tc, feat, hid, w1, b1, w2, b2, v, ctx_out):
    nc = tc.nc
    from contextlib import ExitStack

    with ExitStack() as ctx:
        const = ctx.enter_context(tc.tile_pool(name="const", bufs=1))
        fnat_pool = ctx.enter_context(tc.tile_pool(name="fnat", bufs=9))
        featX_pool = ctx.enter_context(tc.tile_pool(name="featX", bufs=3))
        score_pool = ctx.enter_context(tc.tile_pool(name="score", bufs=3))
        small = ctx.enter_context(tc.tile_pool(name="small", bufs=2))
        att_pool = ctx.enter_context(tc.tile_pool(name="att", bufs=2))
        outst_pool = ctx.enter_context(tc.tile_pool(name="outst", bufs=2))
        # PSUM bank budget (8): trp*3 + scp0 + scp1 + lgp*2(shared w/ atp)
        # + cxp*1 = 8
        pp = ctx.enter_context(tc.tile_pool(name="pp", bufs=1, space="PSUM"))

        NPAIR = BS // 2
        st = {}
        feat_flat = feat.rearrange("b l d -> (b l) d")

        # ---- identity first: the first transposes wait on it, and the
        # gpsimd queue behind it fills with DMA descriptor generation ----
        ident = const.tile([128, 128], F32)
        masks.make_identity(nc, ident[:, :])
        ident_b4 = const.tile([4, 4], BF16)  # bf16 identity for attn transposes
        nc.vector.tensor_copy(ident_b4[:, :], ident[0:4, 0:4])

        def loads(pi):
            # every DMA covers 128 partitions (see module docstring)
            fnats = []
            for half in range(2):
                b = 2 * pi + half
                fnat2 = fnat_pool.tile([128, 2, D], BF16, tag=f"fnat{half}",
                                       name=f"fnat_{pi}_{half}")
                if b < BS - 1:
                    # one DMA for both L-chunks: row p, chunk t reads
                    # feat[b, t*128+p, :] (t=1, p>=97 overreads into the
                    # next batch's rows -- harmless garbage, in bounds)
                    nc.gpsimd.dma_start(
                        fnat2[:, :, :],
                        feat_flat[b * L:b * L + 256, :].rearrange(
                            "(t p) d -> p t d", p=128))
                else:
                    # last batch: chunk B shifted, row p = l 97+p
                    nc.gpsimd.dma_start(fnat2[:, 0, :], feat[b, 0:128, :])
                    nc.gpsimd.dma_start(fnat2[:, 1, :],
                                        feat_flat[BS * L - 128:BS * L, :])
                fnats.append((fnat2[:, 0, :], fnat2[:, 1, :]))
            st[("fnats", pi)] = fnats

        def loads0():
            # pair 0 on the critical path: split fine so the first
            # transposes start as soon as the first slabs land
            fnats = []
            for half in range(2):
                b = half
                fnat2 = fnat_pool.tile([128, 2, D], BF16, tag=f"fnat{half}",
                                       name=f"fnat_0_{half}")
                if half == 0:
                    nc.gpsimd.dma_start(fnat2[:, 0, 0:512], feat[b, 0:128, 0:512])
                    nc.gpsimd.dma_start(fnat2[:, 0, 512:D], feat[b, 0:128, 512:D])
                    nc.gpsimd.dma_start(fnat2[:, 1, :],
                                        feat_flat[b * L + 128:b * L + 256, :])
                else:
                    nc.gpsimd.dma_start(
                        fnat2[:, :, :],
                        feat_flat[b * L:b * L + 256, :].rearrange(
                            "(t p) d -> p t d", p=128))
                fnats.append((fnat2[:, 0, :], fnat2[:, 1, :]))
            st[("fnats", 0)] = fnats

        loads0()

        ones32 = const.tile([1, 32], F32)
        nc.vector.memset(ones32[:, :], 1.0)
        ones32_r = const.tile([1, 32], BF16)
        nc.vector.tensor_copy(ones32_r[:, :], ones32[:, :])

        # ---- W1, pre-permuted to (slab, parity) rows matching featX:
        # partition p, (slab k, parity s) holds W1[256k + 2p + s, :] ----
        w1_sb = const.tile([128, NSLAB, 2, U], BF16)
        nc.scalar.dma_start(
            w1_sb[:, :, :, :],
            w1.rearrange("(k p s) u -> p k s u", p=128, s=2))

        loads(1)

        w2_sb = const.tile([128, 2, U], BF16)   # [h_in_tile, h_tile, u]
        nc.gpsimd.dma_start(w2_sb[:, :, :], w2.rearrange("(k p) u -> p k u", p=128))

        loads(2)

        # ---- V natural [1, 256]: one contiguous descriptor ----
        v_nat = const.tile([1, U], F32)
        nc.scalar.dma_start(v_nat[:, :], v.rearrange("u o -> o u"))

        bsum = const.tile([1, U], BF16)         # b1 + b2 (both added pre-tanh)
        b1_sb = const.tile([1, U], F32)
        b2_sb = const.tile([1, U], F32)
        nc.scalar.dma_start(b1_sb[:, :], b1[None, :])
        nc.scalar.dma_start(b2_sb[:, :], b2[None, :])
        nc.vector.tensor_add(bsum[:, :], b1_sb[:, :], b2_sb[:, :])

        hid_nat = const.tile([32, H], F32)
        nc.scalar.dma_start(hid_nat[:, :], hid[:, :])

        loads(3)
        loads(4)

        hidT = const.tile([128, 2, BS], BF16)   # [h_in_tile, h_tile, b]
        projhT = const.tile([128, 2 * BS], F32)  # [u_in_tile, ut*32+b]
        v_sb = const.tile([128, 2], BF16)       # [u_in_tile, u_tile]
        vq = const.tile([128, 2, 2, 2], BF16)   # [u_in, ut, q2, col]

        def prolog_projh():
            # proj_hT [u, b] = W2.T @ hiddenT + (b1+b2); emitted after
            # head(0) so the PE works on feature transposes while the
            # weight DMAs land. Also builds v_sb / vq from v_nat on-chip.
            vp = pp.tile([128, 512], F32, tag="trp", bufs=3, name="vp")
            for ut in range(2):
                nc.tensor.transpose(
                    vp[:, ut:ut + 1], v_nat[0:1, ut * 128:(ut + 1) * 128],
                    ident[0:1, 0:1])
            nc.vector.tensor_copy(v_sb[:, :], vp[:, 0:2])
            nc.vector.memset(vq[:, :, :, :], 0.0)
            for ut in range(2):
                for q2 in range(2):
                    nc.vector.tensor_copy(vq[:, ut, q2, q2:q2 + 1],
                                          v_sb[:, ut:ut + 1])
            for hk in range(2):
                hp = pp.tile([128, 512], F32, tag="trp", bufs=3)
                nc.tensor.transpose(
                    hp[:, 0:32], hid_nat[0:32, hk * 128:(hk + 1) * 128],
                    ident[0:32, 0:32])
                nc.vector.tensor_copy(hidT[:, hk, :], hp[:, 0:32])
            for ut in range(2):
                php = pp.tile([128, 512], F32, tag="trp", bufs=3)
                for hk in range(2):
                    nc.tensor.matmul(
                        php[:, 0:32],
                        lhsT=w2_sb[:, hk, ut * 128:(ut + 1) * 128],
                        rhs=hidT[:, hk, :],
                        start=(hk == 0), stop=False)
                nc.tensor.matmul(
                    php[:, 0:32],
                    lhsT=bsum[0:1, ut * 128:(ut + 1) * 128],
                    rhs=ones32_r[0:1, :].opt(),
                    start=False, stop=True)
                nc.vector.tensor_copy(projhT[:, ut * BS:(ut + 1) * BS], php[:, 0:32])

        # ---- main loop over batch pairs, software-pipelined ----

        def head0_split():
            # pair 0: process per batch so step-1 on batch 0 starts after
            # only half the pair's bytes (+W1) have landed
            fnats = st[("fnats", 0)]
            featX = featX_pool.tile([128, NSLAB, 2 * L], F32, tag="featX",
                                    name="featX_0")
            frs = [(fA.bitcast(F32), fB.bitcast(F32)) for fA, fB in fnats]
            scps = [pp.tile([128, 512], F32, tag=f"scp{ut}", bufs=1,
                            name=f"scp_0_{ut}") for ut in range(2)]
            featXb = featX[:, :, :].bitcast(BF16).rearrange(
                "p k (l s) -> p k l s", s=2)
            for half in range(2):
                fA32, fB32 = frs[half]
                for k in range(NSLAB):
                    trp = pp.tile([128, 452], F32, tag="trp", bufs=3,
                                  name=f"trp_0_{half}_{k}")
                    nc.tensor.transpose(trp[:, 0:128],
                                        fA32[:, k * 128:(k + 1) * 128],
                                        ident[:, :])
                    nc.tensor.transpose(trp[:, 128:128 + L1 + 1],
                                        fB32[0:L1 + 1, k * 128:(k + 1) * 128],
                                        ident[0:L1 + 1, 0:L1 + 1])
                    nc.vector.tensor_copy(featX[:, k, half * L:(half + 1) * L],
                                          trp[:, 0:L])
                for ut in range(2):
                    for k in range(NSLAB):
                        for s in range(2):
                            nc.tensor.matmul(
                                scps[ut][:, half * L:(half + 1) * L],
                                lhsT=w1_sb[:, k, s, ut * 128:(ut + 1) * 128],
                                rhs=featXb[:, k, half * L:(half + 1) * L, s],
                                start=(k == 0 and s == 0),
                                stop=(k == NSLAB - 1 and s == 1),
                                skip_group_check=True)
            st[("scps", 0)] = scps

        def head(pi):
            fnats = st[("fnats", pi)]
            # packed featT: fp32 lane p of slab k = d-pair (256k+2p, +1);
            # transposes run in f32r mode (1.5 PE cyc/row vs fp32 2.0)
            featX = featX_pool.tile([128, NSLAB, 2 * L], F32, tag="featX",
                                    name=f"featX_{pi}")
            frs = [(fA.bitcast(F32), fB.bitcast(F32)) for fA, fB in fnats]
            last = (pi == NPAIR - 1)
            for k in range(NSLAB):
                trp = pp.tile([128, 452], F32, tag="trp", bufs=3,
                              name=f"trp_{pi}_{k}")
                for half in range(2):
                    fA32, fB32 = frs[half]
                    o = 226 * half
                    nc.tensor.transpose(
                        trp[:, o:o + 128],
                        fA32[:, k * 128:(k + 1) * 128],
                        ident[:, :])
                    if last and half == 1:
                        # shifted chunk B: row p = l 97+p; cols o+97..o+224
                        # overlap chunk A's l 97..127 with identical values
                        nc.tensor.transpose(
                            trp[:, o + 97:o + 225],
                            fB32[:, k * 128:(k + 1) * 128],
                            ident[:, :])
                    else:
                        nc.tensor.transpose(
                            trp[:, o + 128:o + 128 + L1 + 1],
                            fB32[0:L1 + 1, k * 128:(k + 1) * 128],
                            ident[0:L1 + 1, 0:L1 + 1])
                nc.vector.tensor_copy(
                    featX[:, k, :],
                    trp.rearrange("p (h x) -> p h x", h=2)[:, :, 0:L])

            # bf16 parity view: [p, slab, l, s]
            featXb = featX[:, :, :].bitcast(BF16).rearrange(
                "p k (l s) -> p k l s", s=2)
            scps = []
            for ut in range(2):
                scp = pp.tile([128, 512], F32, tag=f"scp{ut}", bufs=1,
                              name=f"scp_{pi}_{ut}")
                for k in range(NSLAB):
                    for s in range(2):
                        nc.tensor.matmul(
                            scp[:, 0:2 * L],
                            lhsT=w1_sb[:, k, s, ut * 128:(ut + 1) * 128],
                            rhs=featXb[:, k, :, s],
                            start=(k == 0 and s == 0),
                            stop=(k == NSLAB - 1 and s == 1))
                scps.append(scp)
            st[("scps", pi)] = scps

        def tanh_part(pi):
            scps = st.pop(("scps", pi))
            score_sb = score_pool.tile([128, 2, 2 * L], BF16, tag="score_sb",
                                       name=f"score_{pi}")
            for ut in range(2):
                for half in range(2):
                    b = 2 * pi + half
                    nc.scalar.activation(
                        score_sb[:, ut, half * L:(half + 1) * L],
                        scps[ut][:, half * L:(half + 1) * L],
                        AF.Tanh,
                        bias=projhT[:, ut * BS + b:ut * BS + b + 1])
            st[("score", pi)] = score_sb

        def logits(pi):
            # rows 0/1 of one psum bank per 2-pair group, accumulated
            q2 = pi % 2
            if q2 == 0:
                lgp = pp.tile([128, 512], F32, tag="lgp", bufs=2,
                              name=f"lgp_{pi // 2}")
                st[("lgp", pi // 2)] = lgp
            else:
                lgp = st[("lgp", pi // 2)]
            score_sb = st.pop(("score", pi))
            for ut in range(2):
                nc.tensor.matmul(
                    lgp[0:2, 0:2 * L],
                    lhsT=vq[:, ut, q2, :],
                    rhs=score_sb[:, ut, :],
                    start=(q2 == 0 and ut == 0), stop=(q2 == 1 and ut == 1),
                    skip_group_check=True)

        def softmax_act(g):
            # exp/recip/scale for batches 4g..4g+3 on ACT+DVE ([2,450] rows)
            lgp = st.pop(("lgp", g))
            expl = small.tile([2, 2 * L], BF16, tag="expl", name=f"expl_{g}")
            esum = small.tile([2, 2], F32, tag="esum", name=f"esum_{g}")
            for half in range(2):
                nc.scalar.activation(
                    expl[0:2, half * L:(half + 1) * L],
                    lgp[0:2, half * L:(half + 1) * L], AF.Exp,
                    accum_out=esum[0:2, half:half + 1])
            rsum = small.tile([2, 2], F32, tag="rsum", name=f"rsum_{g}")
            nc.vector.reciprocal(rsum[:, :], esum[:, :])
            attn = small.tile([2, 2 * L], BF16, tag="attn", name=f"attn_{g}")
            for half in range(2):
                nc.vector.tensor_scalar_mul(
                    attn[0:2, half * L:(half + 1) * L],
                    expl[0:2, half * L:(half + 1) * L],
                    rsum[0:2, half:half + 1])
            st[("attn", g)] = attn

        def transp_flush(g):
            # attn transposes (2 cols per transpose) + ctx flush for the
            # 4 batches of group g
            attn = st.pop(("attn", g))
            last = (g == NPAIR // 2 - 1)
            # atp shares the lgp psum ring: same row bytes (bf16, 2x cols)
            atp = pp.tile([128, 1024], BF16, tag="lgp", bufs=2,
                          name=f"atp_{g}")
            nc.tensor.transpose(atp[0:128, 0:2], attn[0:2, 0:128],
                                ident_b4[0:2, 0:2])
            nc.tensor.transpose(atp[0:128, 2:4], attn[0:2, L:L + 128],
                                ident_b4[0:2, 0:2])
            nc.tensor.transpose(atp[0:97, 4:6], attn[0:2, 128:L],
                                ident_b4[0:2, 0:2])
            if last:
                # b31 (odd row 1) has shifted chunk B: stage a [2,128] row
                # pair -- row 0 (b29) at cols 0:97, row 1 (b31) shifted to
                # cols 31:128 -- then one 2-col transpose covers both.
                a31 = small.tile([2, 128], BF16, tag="a31", name="a31")
                nc.vector.memset(a31[:, :], 0.0)
                nc.vector.tensor_copy(a31[0:2, 31:128], attn[0:2, L + 128:2 * L])
                nc.vector.memset(a31[0:1, 97:128], 0.0)
                nc.vector.tensor_copy(a31[0:1, 0:97], attn[0:1, L + 128:2 * L])
                nc.tensor.transpose(atp[0:128, 6:8], a31[0:2, 0:128],
                                    ident_b4[0:2, 0:2])
            else:
                nc.tensor.transpose(atp[0:97, 6:8], attn[0:2, L + 128:2 * L],
                                    ident_b4[0:2, 0:2])

            attnT = att_pool.tile([128, 4, 64], BF16, tag="attnT",
                                  name=f"attnT_{g}")
            if g < 2:
                # zeros persist across ring reuses: the copies below only
                # ever write cols 0/32 of each 64-col group (rows 0:97 for
                # chunk B), so cells zeroed here are never dirtied
                nc.vector.memset(attnT[:, :, :], 0.0)
            av = attnT.rearrange("p (j e) c -> p j e c", e=2)
            nc.vector.tensor_copy(av[:, :, 0, 0], atp[0:128, 0:2])
            nc.vector.tensor_copy(av[:, :, 1, 0], atp[0:128, 2:4])
            nc.vector.tensor_copy(av[0:97, :, 0, 32], atp[0:97, 4:6])
            if last:
                nc.vector.tensor_copy(av[:, :, 1, 32], atp[0:128, 6:8])
            else:
                nc.vector.tensor_copy(av[0:97, :, 1, 32], atp[0:97, 6:8])

            fn0 = st.pop(("fnats", 2 * g))
            fn1 = st.pop(("fnats", 2 * g + 1))
            group = []
            for j, (fnA, fnB) in enumerate(fn0 + fn1):
                bK = 128 if (last and j == 3) else L1
                group.append((4 * g + j, attnT, j, fnA, fnB, bK))

            # 4 batches -> one PSUM bank at partitions 0/32/64/96.
            out4 = outst_pool.tile([128, D], F32, tag="out_stage",
                                   name=f"outst_{group[0][0]}")
            b0 = group[0][0]
            for doff, dw in ((0, 512), (512, 512), (1024, 256)):
                cxp = pp.tile([128, 512], F32, tag="cxp", bufs=1,
                              name=f"cxp_{group[0][0]}_{doff}")
                for q, (b, attnT_, j, fnatA, fnatB, bK) in enumerate(group):
                    nc.tensor.matmul(
                        cxp[32 * q:32 * q + 32, 0:dw],
                        lhsT=attnT_[0:128, j, 0:32],
                        rhs=fnatA[:, doff:doff + dw],
                        start=True, stop=False,
                        skip_group_check=True,
                        tile_position=(0, 32 * q))
                for q, (b, attnT_, j, fnatA, fnatB, bK) in enumerate(group):
                    nc.tensor.matmul(
                        cxp[32 * q:32 * q + 32, 0:dw],
                        lhsT=attnT_[0:bK, j, 32:64],
                        rhs=fnatB[0:bK, doff:doff + dw],
                        start=False, stop=True,
                        skip_group_check=True,
                        tile_position=(0, 32 * q))
                nc.scalar.copy(out4[0:97, doff:doff + dw],
                               cxp[0:97, 0:dw])
                if last:
                    # final group: store per chunk so the kernel-ending DMA
                    # only covers the last 256 columns (sync queue is idle)
                    nc.sync.dma_start(
                        ctx_out[b0:b0 + 4, doff:doff + dw],
                        out4.rearrange("(q r) d -> q r d", r=32)[:, 0,
                                                                 doff:doff + dw])
            if not last:
                nc.sync.dma_start(
                    ctx_out[b0:b0 + 4, :],
                    out4.rearrange("(q r) d -> q r d", r=32)[:, 0, :])

        for pi in range(NPAIR):
            if pi + 5 < NPAIR:
                loads(pi + 5)
            if pi == 0:
                head0_split()
                prolog_projh()
            else:
                head(pi)
            if pi % 2 == 0 and pi >= 2 and pi < NPAIR - 2:
                # softmax + ctx for the previous group: ACT ops issue
                # before tanh(pi) so the PE transposes/ctx never stall
                softmax_act(pi // 2 - 1)
                transp_flush(pi // 2 - 1)
            if pi == NPAIR - 1:
                # second-to-last group deferred to here: its ctx matmuls
                # keep the PE busy through tanh(15)'s ACT latency
                softmax_act(NPAIR // 2 - 2)
                transp_flush(NPAIR // 2 - 2)
            tanh_part(pi)
            logits(pi)
        softmax_act(NPAIR // 2 - 1)
        transp_flush(NPAIR // 2 - 1)


def _enable_jax_cache():
    try:
        import jax
        jax.config.update("jax_compilation_cache_dir", "/tmp/jax_neff_cache")
        jax.config.update("jax_persistent_cache_min_entry_size_bytes", 0)
        jax.config.update("jax_persistent_cache_min_compile_time_secs", 0)
    except Exception:
        pass


_enable_jax_cache()

_CACHE = {}


def _get_nc():
    if "nc" not in _CACHE:
        _CACHE["nc"] = build_kernel()
    return _CACHE["nc"]


def _run(inputs, trace=False):
    import ml_dtypes
    nc = _get_nc()
    # np.asarray first: harness may pass jax arrays, and jax .astype under
    # the axon platform would trigger a device JIT -- convert on host.
    feat_bf = np.ascontiguousarray(
        np.asarray(inputs["features"]).astype(ml_dtypes.bfloat16))
    w1_bf = np.ascontiguousarray(
        np.asarray(inputs["W1"]).astype(ml_dtypes.bfloat16))
    w2_bf = np.ascontiguousarray(
        np.asarray(inputs["W2"]).astype(ml_dtypes.bfloat16))
    in_maps = []
    for c in range(NCORES):
        sl = slice(c * BS, (c + 1) * BS)
        in_maps.append({
            "features": feat_bf[sl],
            "hidden": np.ascontiguousarray(np.asarray(inputs["hidden"])[sl]),
            "W1": w1_bf,
            "b1": np.ascontiguousarray(np.asarray(inputs["b1"])),
            "W2": w2_bf,
            "b2": np.ascontiguousarray(np.asarray(inputs["b2"])),
            "V": np.ascontiguousarray(np.asarray(inputs["V"])),
            "bV": np.ascontiguousarray(np.asarray(inputs["bV"])),
        })
    res = run_bass_kernel_spmd(nc, in_maps, core_ids=list(range(NCORES)),
                               trace=trace)
    out = np.concatenate([rr["context"] for rr in res.results], axis=0)
    return out, res


def kernel(**inputs):
    out, _ = _run(inputs, trace=False)
    return out
